# revision 11
# baseline (speedup 1.0000x reference)
"""MesoNet Trainium2 kernel: 8-core SPMD.

Device computes the dominant stages (NNConv a21/a11, 4x FCA+LN, trans/xm,
sub1/sub2 big edge-conditioned convs) with nodes + their incoming edges
sharded per core and AllGathers between conv layers.  The small graph-level
AttentiveFP tail (~2% of FLOPs) runs on host.
"""
import numpy as np

N, E, G = 3000, 6000, 150
NP = 3072
S = 384
NC = 8
HID = 160
P = 128

_cache = {}
ACT_PRED = lambda k: k % 2 == 1 and k < 31
SB_BUFS = 3
FCA_GPS = False
AP_GPS = False
CP_GPS = False


# ---------------- host math ----------------
def _sig(v):
    return 1.0 / (1.0 + np.exp(-v))


def _lrelu(v, a=0.01):
    return np.where(v >= 0, v, a * v)


def _elu(v):
    return np.where(v >= 0, v, np.expm1(v))


def _relu(v):
    return np.maximum(v, 0.0)


def _seg_sum(v, seg, n):
    out = np.zeros((n,) + v.shape[1:], np.float32)
    np.add.at(out, seg, v)
    return out


def _seg_softmax(a, seg, n):
    e = np.exp(a)
    s = _seg_sum(e, seg, n)
    return e / np.maximum(s[seg], 1e-16)


def _gru(xv, h, q):
    gi = xv @ q['wih'] + q['bih']
    gh = h @ q['whh'] + q['bhh']
    ir, iz, inn = np.split(gi, 3, -1)
    hr, hz, hn = np.split(gh, 3, -1)
    r = _sig(ir + hr)
    zt = _sig(iz + hz)
    nn_ = np.tanh(inn + r * hn)
    return (1 - zt) * nn_ + zt * h


def _gat(x_src, x_dst, src, dst, q, n_dst):
    hs = x_src @ q['w']
    hd = x_dst @ q['w']
    a = _lrelu((hs @ q['att_src'])[src] + (hd @ q['att_dst'])[dst])
    alpha = _seg_softmax(a, dst, n_dst)
    return _seg_sum(alpha[:, None] * hs[src], dst, n_dst) + q['bias']


def _afp_tail(xc, inter_f, src, dst, batch, edge_attr, p):
    q = p['afp']
    xv = _lrelu(xc @ q['lin1_w'] + q['lin1_b'])
    xj = _lrelu(np.concatenate([xv[src], edge_attr], -1) @ q['gate_lin1'])
    a = _lrelu(xj @ q['gate_att_l'] + (xv @ q['gate_att_r'])[dst])
    alpha = _seg_softmax(a, dst, N)
    hh = _seg_sum((xv[src] @ q['gate_lin2']) * alpha[:, None], dst, N) + q['gate_bias']
    xv = _relu(_gru(_elu(hh), xv, q['gru1']))
    hh = _elu(_gat(xv, xv, src, dst, q['conv1'], N))
    xv = _relu(_gru(hh, xv, q['gru2']))
    out = _relu(_seg_sum(xv, batch, G))
    row = np.arange(N)
    for _ in range(2):
        hh = _elu(_gat(xv, out, row, batch, q['mol_conv'], G))
        out = _relu(_gru(hh, out, q['mol_gru']))
    xg = out @ q['lin2_w'] + q['lin2_b']
    cnt = _seg_sum(np.ones(N, np.float32), batch, G)
    grp_pool = _relu((_seg_sum(inter_f, batch, G) / np.maximum(cnt, 1.0)[:, None])
                     @ p['group_w'] + p['group_b'])
    y = np.concatenate([xg, grp_pool], -1)
    y = _relu(y @ p['fc1_w'] + p['fc1_b'])
    y = _relu(y @ p['fc2_w'] + p['fc2_b'])
    y = _relu(y @ p['fc3_w'] + p['fc3_b'])
    return y @ p['fc4_w'] + p['fc4_b']


def _host_cfc(x, p):
    x2 = x[:, 42:48]
    lt = lambda v: 1.7159 * np.tanh(0.666 * v)
    c = p['cfc']
    h = np.concatenate([x2, x2], -1)
    outs = []
    for _ in range(5):
        z = np.concatenate([x2, h], -1)
        ti = _sig(z @ c['wta'] + c['bta'] + z @ c['wtb'] + c['btb'])
        h = lt(z @ c['wf1'] + c['bf1']) * (1.0 - ti) + ti * lt(z @ c['wf2'] + c['bf2'])
        outs.append(h[:, :6])
    return _relu(np.concatenate(outs, -1) @ p['x22_w'] + p['x22_b'])


# ---------------- device kernel ----------------
def _build(EP, sim=False):
    import concourse.bass as bass
    import concourse.mybir as mybir
    import concourse.tile as tile
    from concourse import bacc
    from concourse.masks import make_identity

    f32 = mybir.dt.float32
    bf16 = mybir.dt.bfloat16
    i32 = mybir.dt.int32
    A = mybir.AluOpType
    AF = mybir.ActivationFunctionType
    X = mybir.AxisListType.X
    NT = EP // P

    def bcast(ap, rep, axis_len):
        """[P, axis_len] -> [P, rep, axis_len] with stride-0 middle dim."""
        return bass.AP(ap.tensor, ap.offset, [ap.ap[0], [0, rep], ap.ap[1]])

    nc = bacc.Bacc("TRN2", target_bir_lowering=False, debug=False, num_devices=NC)

    def din(name, shape, dt=f32):
        return nc.dram_tensor(name, shape, dt, kind="ExternalInput")

    x2o_tab = din("x2o_tab", [NP, 32])
    x1_tab = din("x1_tab", [NP, 41])
    eaT = din("eaT", [11, EP])
    srcg = din("srcg", [P, EP // P], i32)
    dstl = din("dstl", [P, EP // P])
    iota = din("iota", [P, S])
    rdeg = din("rdeg", [P, 3])
    gT = din("gT", [20, S]); x3T = din("x3T", [20, S])
    x1T = din("x1T", [42, S]); x2oT = din("x2oT", [33, S])
    ew1b_a21 = din("ew1b_a21", [11, 32]); ew1b_a11 = din("ew1b_a11", [11, 32])
    W2p_a21 = din("W2p_a21", [33, 1024]); W2p_a11 = din("W2p_a11", [33, 1312])
    rootb_a21 = din("rootb_a21", [33, 32]); rootb_a11 = din("rootb_a11", [42, 32])
    fcaw = {}
    for nm, dq, dkv in [("inter", 19, 19), ("agg", 19, 32), ("aga", 32, 32), ("aae", 32, 32)]:
        fcaw[nm] = dict(
            wq=din(f"fca_{nm}_wq", [dq + 1, 32]), wk=din(f"fca_{nm}_wk", [dkv + 1, 32]),
            wv=din(f"fca_{nm}_wv", [dkv + 1, 32]), wo=din(f"fca_{nm}_wo", [33, 32]))
    transw = din("transw", [97, 96])
    xmw_a = din("xmw_a", [128, 160], bf16)
    xmw_b = din("xmw_b", [33, 160], bf16)
    ew1b_s1 = din("ew1b_s1", [11, 32]); ew1b_s2 = din("ew1b_s2", [11, 32])
    rhsb_s1a = din("rhsb_s1a", [128, 5280], bf16); rhsb_s1b = din("rhsb_s1b", [32, 5280], bf16)
    rhsb_s2a = din("rhsb_s2a", [128, 5280], bf16); rhsb_s2b = din("rhsb_s2b", [32, 5280], bf16)
    rootb_s1a = din("rootb_s1a", [128, 160], bf16); rootb_s1b = din("rootb_s1b", [33, 160], bf16)
    rootb_s2a = din("rootb_s2a", [128, 160], bf16); rootb_s2b = din("rootb_s2b", [33, 160], bf16)

    xc2_out = nc.dram_tensor("xc2", [S, HID], f32, kind="ExternalOutput")
    intf_out = nc.dram_tensor("interf", [S, 32], f32, kind="ExternalOutput")

    with tile.TileContext(nc) as tc:
        with (
            tc.tile_pool(name="sb", bufs=SB_BUFS) as sb,
            tc.tile_pool(name="keep", bufs=1) as kp,
            tc.tile_pool(name="ps", bufs=2, space="PSUM") as ps,
            tc.tile_pool(name="pw", bufs=2, space="PSUM") as pw,
            tc.tile_pool(name="pm", bufs=1, space="PSUM") as pm,
            tc.tile_pool(name="psagg", bufs=1, space="PSUM") as psa,
            tc.tile_pool(name="dram", bufs=1, space="DRAM") as dram,
        ):
            dram_xc0 = dram.tile([S, HID], f32, name="dram_xc0")
            xc_tab = dram.tile([NP, HID], f32, name="xc_tab", addr_space="Shared")
            dram_xc1 = dram.tile([S, HID], f32, name="dram_xc1")
            xc1_tab = dram.tile([NP, HID], f32, name="xc1_tab", addr_space="Shared")

            ident = kp.tile([P, P], f32, name="ident")
            make_identity(nc, ident[:])
            eps_t = kp.tile([P, 1], f32, name="eps_t")
            nc.vector.memset(eps_t[:], 1e-5)
            identb = kp.tile([P, P], bf16, name="identb")
            nc.vector.tensor_copy(identb[:], ident[:])

            def load(t, dt=f32):
                tl = kp.tile(list(t.shape), dt, name="ld_" + t.name)
                nc.sync.dma_start(out=tl[:], in_=t[:, :])
                return tl

            iota_t = load(iota); dstl_t = load(dstl); rdeg_t = load(rdeg)
            eaT_t = load(eaT)
            gT_t = load(gT); x3T_t = load(x3T); x1T_t = load(x1T); x2oT_t = load(x2oT)
            w_a21 = load(ew1b_a21); w_a11 = load(ew1b_a11)
            W2a21 = load(W2p_a21); W2a11 = load(W2p_a11)
            rb21 = load(rootb_a21); rb11 = load(rootb_a11)
            transw_t = load(transw)
            xmw_ta = load(xmw_a, bf16); xmw_tb = load(xmw_b, bf16)
            w_s1 = load(ew1b_s1); w_s2 = load(ew1b_s2)
            rs1a = load(rhsb_s1a, bf16); rs1b = load(rhsb_s1b, bf16)
            rs2a = load(rhsb_s2a, bf16); rs2b = load(rhsb_s2b, bf16)
            rts1a = load(rootb_s1a, bf16); rts1b = load(rootb_s1b, bf16)
            rts2a = load(rootb_s2a, bf16); rts2b = load(rootb_s2b, bf16)
            fcawt = {k: {kk: load(vv) for kk, vv in v.items()} for k, v in fcaw.items()}
            src_t = load(srcg, i32)

            Aps = {}
            for et in range(NT):
                for nb in range(3):
                    ap = kp.tile([P, P], f32, name=f"Ap_{et}_{nb}")
                    (nc.gpsimd if AP_GPS else nc.vector).tensor_scalar(
                        out=ap[:], in0=iota_t[:, nb * P:(nb + 1) * P],
                        scalar1=dstl_t[:, et:et + 1], scalar2=None,
                        op0=A.is_equal)
                    Aps[(et, nb)] = ap

            def transpose_into(dst_ap, src_ap, cols, cast=False):
                pt = ps.tile([P, P], f32, name="tp", tag="tp")
                nc.tensor.transpose(out=pt[0:cols, :], in_=src_ap, identity=ident[:])
                if cast:
                    nc.vector.tensor_copy(dst_ap, pt[0:cols, :])
                else:
                    nc.scalar.copy(dst_ap, pt[0:cols, :])

            # ---------- phase 1: a21 + a11 ----------
            agg_s = [psa.tile([P, 512], f32, name=f"agg{nb}", tag=f"agg{nb}")
                     for nb in range(3)]
            for et in range(NT):
                esl = slice(et * P, (et + 1) * P)
                msgs = sb.tile([P, 64], f32, name="msgs")
                for (wname, W2, cin, co, tab) in [
                        ("a21", W2a21, 32, 0, x2o_tab), ("a11", W2a11, 41, 32, x1_tab)]:
                    wt = w_a21 if wname == "a21" else w_a11
                    h1p = ps.tile([32, P], f32, name="h1p", tag="tp")
                    nc.tensor.matmul(h1p[0:32, :], lhsT=wt[:], rhs=eaT_t[:, esl],
                                     start=True, stop=True)
                    h1T = sb.tile([33, P], f32, name="h1T_" + wname)
                    nc.scalar.activation(h1T[0:32, :], h1p[0:32, :], AF.Relu)
                    nc.vector.memset(h1T[32:33, :], 1.0)
                    xs = sb.tile([P, 48], f32, name="xs_s")
                    nc.gpsimd.indirect_dma_start(
                        out=xs[:, 0:cin], out_offset=None, in_=tab[:, :],
                        in_offset=bass.IndirectOffsetOnAxis(ap=src_t[:, et:et + 1], axis=0))
                    oc = 512 // cin
                    for o0 in range(0, 32, oc):
                        no = min(oc, 32 - o0)
                        wid = no * cin
                        wps = pw.tile([P, 512], f32, name="wps", tag="wps")
                        nc.tensor.matmul(wps[:, 0:wid], lhsT=h1T[:],
                                         rhs=W2[:, o0 * cin:(o0 + no) * cin],
                                         start=True, stop=True)
                        tmp = sb.tile([P, 512], bf16, name="tmp_e")
                        nc.vector.tensor_tensor(
                            out=tmp[:, 0:wid].rearrange("p (o i) -> p o i", i=cin),
                            in0=wps[:, 0:wid].rearrange("p (o i) -> p o i", i=cin),
                            in1=bcast(xs[:, 0:cin], no, cin), op=A.mult)
                        nc.vector.tensor_reduce(
                            out=msgs[:, co + o0:co + o0 + no],
                            in_=tmp[:, 0:wid].rearrange("p (o i) -> p o i", i=cin),
                            axis=X, op=A.add)
                for nb in range(3):
                    nc.tensor.matmul(agg_s[nb][:, 0:64], lhsT=Aps[(et, nb)][:],
                                     rhs=msgs[:], start=(et == 0), stop=(et == NT - 1))

            x1v = {}; x2v = {}; x1vT = {}; x2vT = {}
            for nb in range(3):
                nsl = slice(nb * P, (nb + 1) * P)
                nc.tensor.matmul(agg_s[nb][:, 64:96], lhsT=x2oT_t[:, nsl], rhs=rb21[:],
                                 start=True, stop=True)
                nc.tensor.matmul(agg_s[nb][:, 96:128], lhsT=x1T_t[:, nsl], rhs=rb11[:],
                                 start=True, stop=True)
                for (c0, r0, dd) in [(0, 64, x2v), (32, 96, x1v)]:
                    sA = sb.tile([P, 32], f32, name="sA")
                    nc.vector.tensor_scalar(out=sA[:], in0=agg_s[nb][:, c0:c0 + 32],
                                            scalar1=rdeg_t[:, nb:nb + 1], scalar2=None,
                                            op0=A.mult)
                    vv = kp.tile([P, 32], f32, name=f"v{r0}_{nb}")
                    nc.vector.tensor_tensor(out=vv[:], in0=sA[:],
                                            in1=agg_s[nb][:, r0:r0 + 32], op=A.add)
                    nc.vector.tensor_scalar(out=vv[:], in0=vv[:], scalar1=0.0,
                                            scalar2=None, op0=A.max)
                    dd[nb] = vv
                for (dd, dt_, pref) in [(x2v, x2vT, "t2v"), (x1v, x1vT, "t1v")]:
                    tt = kp.tile([33, P], f32, name=f"{pref}_{nb}")
                    transpose_into(tt[0:32, :], dd[nb][:], 32)
                    nc.vector.memset(tt[32:33, :], 1.0)
                    dt_[nb] = tt

            # ---------- phase 2: FCA chain ----------
            def fca_tile(qT_ap, kvT_ap, w, nb, name):
                qkv = psa.tile([P, 96], f32, name=f"qkv_{name}_{nb}", tag=f"agg{nb}")
                nc.tensor.matmul(qkv[:, 0:32], lhsT=qT_ap, rhs=w['wq'][:], start=True, stop=True)
                nc.tensor.matmul(qkv[:, 32:64], lhsT=kvT_ap, rhs=w['wk'][:], start=True, stop=True)
                nc.tensor.matmul(qkv[:, 64:96], lhsT=kvT_ap, rhs=w['wv'][:], start=True, stop=True)
                q_s = sb.tile([P, 32], f32, name="q_s")
                k_s = sb.tile([P, 32], f32, name="k_s")
                v_s = sb.tile([P, 32], bf16, name="v_s")
                nc.scalar.copy(q_s[:], qkv[:, 0:32])
                nc.scalar.copy(k_s[:], qkv[:, 32:64])
                nc.vector.tensor_copy(v_s[:], qkv[:, 64:96])
                lg = sb.tile([P, 1024], f32, name="lg")
                qb = bass.AP(q_s[:].tensor, q_s[:].offset,
                             [q_s[:].ap[0], [1, 32], [0, 32]])
                (nc.gpsimd if FCA_GPS else nc.vector).tensor_tensor(
                    out=lg[:].rearrange("p (i j) -> p i j", j=32),
                    in0=qb, in1=bcast(k_s[:, 0:32], 32, 32), op=A.mult)
                ee = sb.tile([P, 1024], bf16, name="ee")
                nc.scalar.activation(ee[:], lg[:], AF.Exp)
                ev = sb.tile([P, 1024], bf16, name="ev")
                (nc.gpsimd if FCA_GPS else nc.vector).tensor_tensor(
                    out=ev[:].rearrange("p (i j) -> p i j", j=32),
                    in0=ee[:].rearrange("p (i j) -> p i j", j=32),
                    in1=bcast(v_s[:, 0:32], 32, 32), op=A.mult)
                o_u = sb.tile([P, 32], f32, name="o_u")
                s_u = sb.tile([P, 32], f32, name="s_u")
                nc.vector.tensor_reduce(out=o_u[:], in_=ev[:].rearrange("p (i j) -> p i j", j=32),
                                        axis=X, op=A.add)
                nc.vector.tensor_reduce(out=s_u[:], in_=ee[:].rearrange("p (i j) -> p i j", j=32),
                                        axis=X, op=A.add)
                rs = sb.tile([P, 32], f32, name="rs")
                nc.vector.reciprocal(rs[:], s_u[:])
                oo = sb.tile([P, 32], f32, name="oo")
                nc.vector.tensor_tensor(out=oo[:], in0=o_u[:], in1=rs[:], op=A.mult)
                ooT = sb.tile([33, P], f32, name="ooT")
                transpose_into(ooT[0:32, :], oo[:], 32)
                nc.vector.memset(ooT[32:33, :], 1.0)
                op_ = pw.tile([P, 512], f32, name="wps", tag="wps")
                nc.tensor.matmul(op_[:, 0:32], lhsT=ooT[:], rhs=w['wo'][:], start=True, stop=True)
                z = sb.tile([P, 32], f32, name="z_ln")
                nc.vector.tensor_tensor(out=z[:], in0=q_s[:], in1=op_[:, 0:32], op=A.add)
                mu = sb.tile([P, 1], f32, name="mu")
                nc.vector.tensor_reduce(out=mu[:], in_=z[:], axis=X, op=A.add)
                nc.vector.tensor_scalar(out=mu[:], in0=mu[:], scalar1=1.0 / 32,
                                        scalar2=None, op0=A.mult)
                zc = sb.tile([P, 32], f32, name="zc")
                nc.vector.tensor_scalar(out=zc[:], in0=z[:], scalar1=mu[:, 0:1],
                                        scalar2=None, op0=A.subtract)
                junk = sb.tile([P, 32], bf16, name="junk")
                vs = sb.tile([P, 1], f32, name="vs")
                nc.scalar.activation(junk[:], zc[:], AF.Square, accum_out=vs[:])
                lnv = sb.tile([P, 1], f32, name="lnv")
                nc.scalar.activation(lnv[:], vs[:], AF.Ln, scale=1.0 / 32, bias=eps_t[:, 0:1])
                rsq = sb.tile([P, 1], f32, name="rsq")
                nc.scalar.activation(rsq[:], lnv[:], AF.Exp, scale=-0.5)
                out = kp.tile([P, 32], f32, name=f"fca_{name}_{nb}")
                nc.vector.tensor_scalar(out=out[:], in0=zc[:], scalar1=rsq[:, 0:1],
                                        scalar2=None, op0=A.mult)
                outT = kp.tile([33, P], f32, name=f"fcaT_{name}_{nb}")
                transpose_into(outT[0:32, :], out[:], 32)
                nc.vector.memset(outT[32:33, :], 1.0)
                return out, outT

            xcT_a = {}; xcT_b = {}
            for nb in range(3):
                nsl = slice(nb * P, (nb + 1) * P)
                inter_o, interT = fca_tile(gT_t[:, nsl], gT_t[:, nsl], fcawt['inter'], nb, "in")
                gu_o, guT = fca_tile(x3T_t[:, nsl], interT[:], fcawt['agg'], nb, "gu")
                grp_o, grpT = fca_tile(x1vT[nb][:], guT[:], fcawt['aga'], nb, "gr")
                au_o, auT = fca_tile(x2vT[nb][:], grpT[:], fcawt['aae'], nb, "au")
                nc.sync.dma_start(out=intf_out[nsl, :], in_=inter_o[:])
                catT = sb.tile([97, P], f32, name="catT")
                (nc.gpsimd if CP_GPS else nc.vector).tensor_copy(catT[0:32, :], guT[0:32, :])
                (nc.gpsimd if CP_GPS else nc.vector).tensor_copy(catT[32:64, :], grpT[0:32, :])
                (nc.gpsimd if CP_GPS else nc.vector).tensor_copy(catT[64:96, :], auT[0:32, :])
                nc.vector.memset(catT[96:97, :], 1.0)
                xxp = pw.tile([P, 512], f32, name="wps", tag="wps")
                nc.tensor.matmul(xxp[:, 0:96], lhsT=catT[:], rhs=transw_t[:],
                                 start=True, stop=True)
                xx = sb.tile([P, 96], f32, name="xx")
                nc.scalar.activation(xx[:], xxp[:, 0:96], AF.Relu)
                xcat_a = sb.tile([P, P], bf16, name="xcat_a")
                xcat_b = sb.tile([33, P], bf16, name="xcat_b")
                (nc.gpsimd if CP_GPS else nc.vector).tensor_copy(xcat_a[0:32, :], x1vT[nb][0:32, :])
                (nc.gpsimd if CP_GPS else nc.vector).tensor_copy(xcat_a[32:64, :], x2vT[nb][0:32, :])
                ptx = ps.tile([P, P], f32, name="tp", tag="tp")
                nc.tensor.transpose(out=ptx[0:96, :], in_=xx[:], identity=ident[:])
                nc.vector.tensor_copy(xcat_a[64:128, :], ptx[0:64, :])
                nc.vector.tensor_copy(xcat_b[0:32, :], ptx[64:96, :])
                nc.vector.memset(xcat_b[32:33, :], 1.0)
                xcp = pw.tile([P, 512], f32, name="wps", tag="wps")
                nc.tensor.matmul(xcp[:, 0:160], lhsT=xcat_a[:], rhs=xmw_ta[:],
                                 start=True, stop=False)
                nc.tensor.matmul(xcp[:, 0:160], lhsT=xcat_b[:], rhs=xmw_tb[:],
                                 start=False, stop=True)
                xc = sb.tile([P, HID], f32, name="xc")
                nc.scalar.activation(xc[:], xcp[:, 0:160], AF.Relu)
                xa = kp.tile([P, P], bf16, name=f"xcTa{nb}")
                xb = kp.tile([33, P], bf16, name=f"xcTb{nb}")
                transpose_into(xa[:], xc[:, 0:128], 128, cast=True)
                transpose_into(xb[0:32, :], xc[:, 128:160], 32, cast=True)
                nc.vector.memset(xb[32:33, :], 1.0)
                xcT_a[nb] = xa; xcT_b[nb] = xb
                nc.sync.dma_start(out=dram_xc0[nsl, :], in_=xc[:])

            if sim:
                nc.sync.dma_start(out=xc_tab[0:S, :], in_=dram_xc0[:, :])
            else:
                nc.gpsimd.collective_compute(
                    "AllGather", A.bypass, replica_groups=[list(range(NC))],
                    ins=[dram_xc0[:, :]], outs=[xc_tab[:, :]])

            # ---------- big convs ----------
            def bigconv(tab, w_e, rs_a, rs_b, rtwa, rtwb, xTa, xTb, stage):
                agg = [psa.tile([P, 512], f32, name=f"agg{nb}_{stage}", tag=f"agg{nb}")
                       for nb in range(3)]
                for et in range(NT):
                    esl = slice(et * P, (et + 1) * P)
                    h1p = ps.tile([P, 96], f32, name="h1pe", tag="tp")
                    nc.tensor.matmul(h1p[:, 0:32], lhsT=eaT_t[:, esl], rhs=w_e[:],
                                     start=True, stop=True)
                    h1 = sb.tile([P, 33], f32, name="h1e")
                    nc.scalar.activation(h1[:, 0:32], h1p[:, 0:32], AF.Relu)
                    nc.vector.memset(h1[:, 32:33], 1.0)
                    xs = sb.tile([P, HID], f32, name="xs_b")
                    nc.gpsimd.indirect_dma_start(
                        out=xs[:], out_offset=None, in_=tab[:, :],
                        in_offset=bass.IndirectOffsetOnAxis(ap=src_t[:, et:et + 1], axis=0))
                    xsTa = sb.tile([P, P], bf16, name="xsTa")
                    xsTb = sb.tile([32, P], bf16, name="xsTb")
                    transpose_into(xsTa[:], xs[:, 0:128], 128, cast=True)
                    transpose_into(xsTb[0:32, :], xs[:, 128:160], 32, cast=True)
                    msg = sb.tile([P, HID], f32, name="msg_b")
                    msgp = pm.tile([P, HID], f32, name="msgp", tag="msgp")
                    nacts = 0
                    for kc in range(11):
                        c0 = kc * 480
                        tps = pw.tile([P, 512], f32, name="wps", tag="wps")
                        nc.tensor.matmul(tps[:, 0:480], lhsT=xsTa[:],
                                         rhs=rs_a[:, c0:c0 + 480], start=True, stop=False)
                        nc.tensor.matmul(tps[:, 0:480], lhsT=xsTb[:],
                                         rhs=rs_b[:, c0:c0 + 480], start=False, stop=True)
                        for j in range(3):
                            k = kc * 3 + j
                            if k == 0:
                                nc.vector.tensor_scalar(
                                    out=msg[:], in0=tps[:, 0:160],
                                    scalar1=h1[:, 0:1], scalar2=None, op0=A.mult)
                            elif ACT_PRED(k):
                                tmpk = sb.tile([P, HID], bf16, name="tmpk")
                                nc.scalar.activation(tmpk[:], tps[:, j * 160:(j + 1) * 160],
                                                     AF.Copy, scale=h1[:, k:k + 1])
                                nc.tensor.matmul(msgp[:], lhsT=identb[:], rhs=tmpk[:],
                                                 start=(nacts == 0), stop=False)
                                nacts += 1
                            else:
                                nc.vector.scalar_tensor_tensor(
                                    out=msg[:], in0=tps[:, j * 160:(j + 1) * 160],
                                    scalar=h1[:, k:k + 1], in1=msg[:],
                                    op0=A.mult, op1=A.add)
                    msg2 = sb.tile([P, HID], f32, name="msg2")
                    nc.vector.tensor_tensor(out=msg2[:], in0=msgp[:], in1=msg[:], op=A.add)
                    for nb in range(3):
                        nc.tensor.matmul(agg[nb][:, 0:160], lhsT=Aps[(et, nb)][:],
                                         rhs=msg2[:], start=(et == 0), stop=(et == NT - 1))
                outs = []
                for nb in range(3):
                    nc.tensor.matmul(agg[nb][:, 160:320], lhsT=xTa[nb][:],
                                     rhs=rtwa[:], start=True, stop=False)
                    nc.tensor.matmul(agg[nb][:, 160:320], lhsT=xTb[nb][:],
                                     rhs=rtwb[:], start=False, stop=True)
                    sA = sb.tile([P, HID], f32, name="sAb")
                    nc.vector.tensor_scalar(out=sA[:], in0=agg[nb][:, 0:160],
                                            scalar1=rdeg_t[:, nb:nb + 1], scalar2=None,
                                            op0=A.mult)
                    oo = sb.tile([P, HID], f32, name="oo_b")
                    nc.vector.tensor_tensor(out=oo[:], in0=sA[:],
                                            in1=agg[nb][:, 160:320], op=A.add)
                    o = kp.tile([P, HID], f32, name=f"xcs_{stage}_{nb}")
                    nc.scalar.activation(o[:], oo[:], AF.Relu)
                    outs.append(o)
                return outs

            xc1o = bigconv(xc_tab, w_s1, rs1a, rs1b, rts1a, rts1b, xcT_a, xcT_b, "s1")
            xc1T_a = {}; xc1T_b = {}
            for nb in range(3):
                nc.sync.dma_start(out=dram_xc1[nb * P:(nb + 1) * P, :], in_=xc1o[nb][:])
                xa = kp.tile([P, P], bf16, name=f"x1Ta{nb}")
                xb = kp.tile([33, P], bf16, name=f"x1Tb{nb}")
                transpose_into(xa[:], xc1o[nb][:, 0:128], 128, cast=True)
                transpose_into(xb[0:32, :], xc1o[nb][:, 128:160], 32, cast=True)
                nc.vector.memset(xb[32:33, :], 1.0)
                xc1T_a[nb] = xa; xc1T_b[nb] = xb
            if sim:
                nc.sync.dma_start(out=xc1_tab[0:S, :], in_=dram_xc1[:, :])
            else:
                nc.gpsimd.collective_compute(
                    "AllGather", A.bypass, replica_groups=[list(range(NC))],
                    ins=[dram_xc1[:, :]], outs=[xc1_tab[:, :]])

            xc2o = bigconv(xc1_tab, w_s2, rs2a, rs2b, rts2a, rts2b, xc1T_a, xc1T_b, "s2")
            for nb in range(3):
                nc.sync.dma_start(out=xc2_out[nb * P:(nb + 1) * P, :], in_=xc2o[nb][:])

    nc.compile()
    return nc


def _prep(x, edge_index, edge_attr, batch, params):
    import ml_dtypes
    p = params
    src, dst = np.asarray(edge_index[0]), np.asarray(edge_index[1])
    x = np.asarray(x, np.float32)
    ea_np = np.asarray(edge_attr, np.float32)
    x2o = _host_cfc(x, p)

    order = np.argsort(dst, kind='stable')
    ssrc, sdst, sea = src[order], dst[order], ea_np[order]

    shards = []
    maxe = 0
    for s in range(NC):
        lo, hi = s * S, (s + 1) * S
        m = (sdst >= lo) & (sdst < hi)
        shards.append((ssrc[m], (sdst[m] - lo).astype(np.int64), sea[m]))
        maxe = max(maxe, int(m.sum()))
    EP = ((maxe + P - 1) // P) * P

    key = ("nc", EP)
    if key not in _cache:
        _cache[key] = _build(EP)
    nc = _cache[key]

    def pad_rows(a, n, fill=0.0):
        out = np.full((n,) + a.shape[1:], fill, np.float32)
        out[:a.shape[0]] = a
        return out

    xf = pad_rows(x, NP)
    x2of = pad_rows(np.asarray(x2o, np.float32), NP)
    x1f = np.ascontiguousarray(xf[:, :41])

    def aug(w, b):
        return np.concatenate([np.asarray(w, np.float32),
                               np.asarray(b, np.float32)[None, :]], 0)

    def perm_small(pp, cin):
        ew2 = np.asarray(pp['ew2'], np.float32)
        eb2 = np.asarray(pp['eb2'], np.float32)
        W = np.empty((33, 32 * cin), np.float32)
        w3 = ew2.reshape(32, cin, 32)
        W[0:32] = np.transpose(w3, (0, 2, 1)).reshape(32, 32 * cin)
        W[32] = eb2.reshape(cin, 32).T.reshape(-1)
        return W

    def perm_big(pp):
        ew2 = np.asarray(pp['ew2'], np.float32).reshape(32, HID, HID)
        eb2 = np.asarray(pp['eb2'], np.float32).reshape(HID, HID)
        R = np.empty((HID, 33 * HID), np.float32)
        for k in range(32):
            R[:, k * HID:(k + 1) * HID] = ew2[k]
        R[:, 32 * HID:] = eb2
        return R

    fca_in = {}
    for nm in ['inter', 'agg', 'aga', 'aae']:
        w = p[nm]
        fca_in[f"fca_{nm}_wq"] = aug(w['wq'], w['bq'])
        fca_in[f"fca_{nm}_wk"] = aug(w['wk'], w['bk'])
        fca_in[f"fca_{nm}_wv"] = aug(w['wv'], w['bv'])
        fca_in[f"fca_{nm}_wo"] = aug(w['wo'], w['bo'])

    rb_s1 = perm_big(p['sub1']); rb_s2 = perm_big(p['sub2'])
    bf = ml_dtypes.bfloat16

    in_maps = []
    for s in range(NC):
        es, ed, ea = shards[s]
        ne = len(es)
        srcg_f = np.zeros(EP, np.int32); srcg_f[:ne] = es
        srcg = np.ascontiguousarray(srcg_f.reshape(EP // P, P).T)
        dstl_f = np.full(EP, -1000.0, np.float32); dstl_f[:ne] = ed
        dstl = np.ascontiguousarray(dstl_f.reshape(EP // P, P).T)
        eaT_in = np.ones((11, EP), np.float32)
        eaT_in[0:10, :ne] = ea.T
        eaT_in[0:10, ne:] = 0.0
        deg = np.bincount(ed, minlength=S)[:S].astype(np.float32)
        rdeg_in = (1.0 / np.maximum(deg, 1.0)).reshape(3, P).T.copy()
        lo = s * S
        xo = xf[lo:lo + S]
        ones_row = np.ones((1, S), np.float32)
        m = dict(
            x2o_tab=x2of, x1_tab=x1f, eaT=eaT_in, srcg=srcg, dstl=dstl,
            iota=np.broadcast_to(np.arange(S, dtype=np.float32)[None, :], (P, S)).copy(),
            rdeg=rdeg_in,
            gT=np.concatenate([xo[:, 67:86].T, ones_row], 0),
            x3T=np.concatenate([xo[:, 48:67].T, ones_row], 0),
            x1T=np.concatenate([xo[:, 0:41].T, ones_row], 0),
            x2oT=np.concatenate([x2of[lo:lo + S].T, ones_row], 0),
            ew1b_a21=aug(p['a21']['ew1'], p['a21']['eb1']),
            ew1b_a11=aug(p['a11']['ew1'], p['a11']['eb1']),
            W2p_a21=perm_small(p['a21'], 32), W2p_a11=perm_small(p['a11'], 41),
            rootb_a21=aug(p['a21']['root'], p['a21']['bias']),
            rootb_a11=aug(p['a11']['root'], p['a11']['bias']),
            transw=aug(p['trans_w'], p['trans_b']),
            xmw_a=aug(p['xm_w'], p['xm_b'])[0:128].astype(bf),
            xmw_b=aug(p['xm_w'], p['xm_b'])[128:161].astype(bf),
            ew1b_s1=aug(p['sub1']['ew1'], p['sub1']['eb1']),
            ew1b_s2=aug(p['sub2']['ew1'], p['sub2']['eb1']),
            rhsb_s1a=rb_s1[0:128].astype(bf), rhsb_s1b=rb_s1[128:160].astype(bf),
            rhsb_s2a=rb_s2[0:128].astype(bf), rhsb_s2b=rb_s2[128:160].astype(bf),
            rootb_s1a=aug(p['sub1']['root'], p['sub1']['bias'])[0:128].astype(bf),
            rootb_s1b=aug(p['sub1']['root'], p['sub1']['bias'])[128:161].astype(bf),
            rootb_s2a=aug(p['sub2']['root'], p['sub2']['bias'])[0:128].astype(bf),
            rootb_s2b=aug(p['sub2']['root'], p['sub2']['bias'])[128:161].astype(bf),
            **{k: v.copy() for k, v in fca_in.items()},
        )
        in_maps.append(m)
    return nc, in_maps, ea_np


def kernel(x, edge_index, edge_attr, batch, params):
    import time
    from concourse.bass_utils import run_bass_kernel_spmd
    nc, in_maps, ea_np = _prep(x, edge_index, edge_attr, batch, params)
    t0 = time.time()
    res = run_bass_kernel_spmd(nc, in_maps, core_ids=list(range(NC)))
    dt_ns = int((time.time() - t0) * 1e9)
    _cache['exec_ns'] = min(_cache.get('exec_ns', 1 << 62), dt_ns)
    xc2 = np.concatenate([res.results[s]["xc2"] for s in range(NC)], 0)[:N]
    intf = np.concatenate([res.results[s]["interf"] for s in range(NC)], 0)[:N]
    src, dst = np.asarray(edge_index[0]), np.asarray(edge_index[1])
    y = _afp_tail(xc2, intf, src, dst, np.asarray(batch), ea_np, params)
    return y.astype(np.float32)


# revision 14
# speedup vs baseline: 1.0268x; 1.0268x over previous
"""MesoNet Trainium2 kernel: 8-core SPMD.

Device computes the dominant stages (NNConv a21/a11, 4x FCA+LN, trans/xm,
sub1/sub2 big edge-conditioned convs) with nodes + their incoming edges
sharded per core and AllGathers between conv layers.  The small graph-level
AttentiveFP tail (~2% of FLOPs) runs on host.
"""
import numpy as np

N, E, G = 3000, 6000, 150
NP = 3072
S = 384
NC = 8
HID = 160
P = 128

_cache = {}
ACT_PRED = lambda k: k % 2 == 1 and k < 31
SB_BUFS = 3
FCA_GPS = False
AP_GPS = False
CP_GPS = False


# ---------------- host math ----------------
def _sig(v):
    return 1.0 / (1.0 + np.exp(-v))


def _lrelu(v, a=0.01):
    return np.where(v >= 0, v, a * v)


def _elu(v):
    return np.where(v >= 0, v, np.expm1(v))


def _relu(v):
    return np.maximum(v, 0.0)


def _seg_sum(v, seg, n):
    out = np.zeros((n,) + v.shape[1:], np.float32)
    np.add.at(out, seg, v)
    return out


def _seg_softmax(a, seg, n):
    e = np.exp(a)
    s = _seg_sum(e, seg, n)
    return e / np.maximum(s[seg], 1e-16)


def _gru(xv, h, q):
    gi = xv @ q['wih'] + q['bih']
    gh = h @ q['whh'] + q['bhh']
    ir, iz, inn = np.split(gi, 3, -1)
    hr, hz, hn = np.split(gh, 3, -1)
    r = _sig(ir + hr)
    zt = _sig(iz + hz)
    nn_ = np.tanh(inn + r * hn)
    return (1 - zt) * nn_ + zt * h


def _gat(x_src, x_dst, src, dst, q, n_dst):
    hs = x_src @ q['w']
    hd = x_dst @ q['w']
    a = _lrelu((hs @ q['att_src'])[src] + (hd @ q['att_dst'])[dst])
    alpha = _seg_softmax(a, dst, n_dst)
    return _seg_sum(alpha[:, None] * hs[src], dst, n_dst) + q['bias']


def _afp_tail(xc, inter_f, src, dst, batch, edge_attr, p):
    q = p['afp']
    xv = _lrelu(xc @ q['lin1_w'] + q['lin1_b'])
    xj = _lrelu(np.concatenate([xv[src], edge_attr], -1) @ q['gate_lin1'])
    a = _lrelu(xj @ q['gate_att_l'] + (xv @ q['gate_att_r'])[dst])
    alpha = _seg_softmax(a, dst, N)
    hh = _seg_sum((xv[src] @ q['gate_lin2']) * alpha[:, None], dst, N) + q['gate_bias']
    xv = _relu(_gru(_elu(hh), xv, q['gru1']))
    hh = _elu(_gat(xv, xv, src, dst, q['conv1'], N))
    xv = _relu(_gru(hh, xv, q['gru2']))
    out = _relu(_seg_sum(xv, batch, G))
    row = np.arange(N)
    for _ in range(2):
        hh = _elu(_gat(xv, out, row, batch, q['mol_conv'], G))
        out = _relu(_gru(hh, out, q['mol_gru']))
    xg = out @ q['lin2_w'] + q['lin2_b']
    cnt = _seg_sum(np.ones(N, np.float32), batch, G)
    grp_pool = _relu((_seg_sum(inter_f, batch, G) / np.maximum(cnt, 1.0)[:, None])
                     @ p['group_w'] + p['group_b'])
    y = np.concatenate([xg, grp_pool], -1)
    y = _relu(y @ p['fc1_w'] + p['fc1_b'])
    y = _relu(y @ p['fc2_w'] + p['fc2_b'])
    y = _relu(y @ p['fc3_w'] + p['fc3_b'])
    return y @ p['fc4_w'] + p['fc4_b']


def _host_cfc(x, p):
    x2 = x[:, 42:48]
    lt = lambda v: 1.7159 * np.tanh(0.666 * v)
    c = p['cfc']
    h = np.concatenate([x2, x2], -1)
    outs = []
    for _ in range(5):
        z = np.concatenate([x2, h], -1)
        ti = _sig(z @ c['wta'] + c['bta'] + z @ c['wtb'] + c['btb'])
        h = lt(z @ c['wf1'] + c['bf1']) * (1.0 - ti) + ti * lt(z @ c['wf2'] + c['bf2'])
        outs.append(h[:, :6])
    return _relu(np.concatenate(outs, -1) @ p['x22_w'] + p['x22_b'])


# ---------------- device kernel ----------------
def _build(EP, sim=False):
    import concourse.bass as bass
    import concourse.mybir as mybir
    import concourse.tile as tile
    from concourse import bacc
    from concourse.masks import make_identity

    f32 = mybir.dt.float32
    bf16 = mybir.dt.bfloat16
    i32 = mybir.dt.int32
    A = mybir.AluOpType
    AF = mybir.ActivationFunctionType
    X = mybir.AxisListType.X
    NT = EP // P

    def bcast(ap, rep, axis_len):
        """[P, axis_len] -> [P, rep, axis_len] with stride-0 middle dim."""
        return bass.AP(ap.tensor, ap.offset, [ap.ap[0], [0, rep], ap.ap[1]])

    nc = bacc.Bacc("TRN2", target_bir_lowering=False, debug=False, num_devices=NC)

    def din(name, shape, dt=f32):
        return nc.dram_tensor(name, shape, dt, kind="ExternalInput")

    x2o_tab = din("x2o_tab", [NP, 32])
    x1_tab = din("x1_tab", [NP, 41])
    eaT = din("eaT", [11, EP])
    srcg = din("srcg", [P, EP // P], i32)
    dstl = din("dstl", [P, EP // P])
    iota = din("iota", [P, S])
    rdeg = din("rdeg", [P, 3])
    gT = din("gT", [20, S]); x3T = din("x3T", [20, S])
    x1T = din("x1T", [42, S]); x2oT = din("x2oT", [33, S])
    ew1b_a21 = din("ew1b_a21", [11, 32]); ew1b_a11 = din("ew1b_a11", [11, 32])
    W2p_a21 = din("W2p_a21", [33, 1024]); W2p_a11 = din("W2p_a11", [33, 1312])
    rootb_a21 = din("rootb_a21", [33, 32]); rootb_a11 = din("rootb_a11", [42, 32])
    fcaw = {}
    for nm, dq, dkv in [("inter", 19, 19), ("agg", 19, 32), ("aga", 32, 32), ("aae", 32, 32)]:
        fcaw[nm] = dict(
            wq=din(f"fca_{nm}_wq", [dq + 1, 32]), wk=din(f"fca_{nm}_wk", [dkv + 1, 32]),
            wv=din(f"fca_{nm}_wv", [dkv + 1, 32]), wo=din(f"fca_{nm}_wo", [33, 32]))
    transw = din("transw", [97, 96])
    xmw_a = din("xmw_a", [128, 160], bf16)
    xmw_b = din("xmw_b", [33, 160], bf16)
    ew1b_s1 = din("ew1b_s1", [11, 32]); ew1b_s2 = din("ew1b_s2", [11, 32])
    rhsb_s1a = din("rhsb_s1a", [128, 5280], bf16); rhsb_s1b = din("rhsb_s1b", [32, 5280], bf16)
    rhsb_s2a = din("rhsb_s2a", [128, 5280], bf16); rhsb_s2b = din("rhsb_s2b", [32, 5280], bf16)
    rootb_s1a = din("rootb_s1a", [128, 160], bf16); rootb_s1b = din("rootb_s1b", [33, 160], bf16)
    rootb_s2a = din("rootb_s2a", [128, 160], bf16); rootb_s2b = din("rootb_s2b", [33, 160], bf16)

    xc2_out = nc.dram_tensor("xc2", [S, HID], f32, kind="ExternalOutput")
    intf_out = nc.dram_tensor("interf", [S, 32], f32, kind="ExternalOutput")

    with tile.TileContext(nc) as tc:
        with (
            tc.tile_pool(name="sb", bufs=SB_BUFS) as sb,
            tc.tile_pool(name="keep", bufs=1) as kp,
            tc.tile_pool(name="ps", bufs=2, space="PSUM") as ps,
            tc.tile_pool(name="pw", bufs=2, space="PSUM") as pw,
            tc.tile_pool(name="pm", bufs=1, space="PSUM") as pm,
            tc.tile_pool(name="psagg", bufs=1, space="PSUM") as psa,
            tc.tile_pool(name="dram", bufs=1, space="DRAM") as dram,
        ):
            dram_xc0 = dram.tile([S, HID], f32, name="dram_xc0")
            xc_tab = dram.tile([NP, HID], f32, name="xc_tab", addr_space="Shared")
            dram_xc1 = dram.tile([S, HID], f32, name="dram_xc1")
            xc1_tab = dram.tile([NP, HID], f32, name="xc1_tab", addr_space="Shared")

            ident = kp.tile([P, P], f32, name="ident")
            make_identity(nc, ident[:])
            eps_t = kp.tile([P, 1], f32, name="eps_t")
            nc.vector.memset(eps_t[:], 1e-5)
            identb = kp.tile([P, P], bf16, name="identb")
            nc.vector.tensor_copy(identb[:], ident[:])

            def load(t, dt=f32):
                tl = kp.tile(list(t.shape), dt, name="ld_" + t.name)
                nc.sync.dma_start(out=tl[:], in_=t[:, :])
                return tl

            iota_t = load(iota); dstl_t = load(dstl); rdeg_t = load(rdeg)
            eaT_t = load(eaT)
            gT_t = load(gT); x3T_t = load(x3T); x1T_t = load(x1T); x2oT_t = load(x2oT)
            w_a21 = load(ew1b_a21); w_a11 = load(ew1b_a11)
            W2a21 = load(W2p_a21); W2a11 = load(W2p_a11)
            rb21 = load(rootb_a21); rb11 = load(rootb_a11)
            transw_t = load(transw)
            xmw_ta = load(xmw_a, bf16); xmw_tb = load(xmw_b, bf16)
            w_s1 = load(ew1b_s1); w_s2 = load(ew1b_s2)
            rs1a = load(rhsb_s1a, bf16); rs1b = load(rhsb_s1b, bf16)
            rs2a = load(rhsb_s2a, bf16); rs2b = load(rhsb_s2b, bf16)
            rts1a = load(rootb_s1a, bf16); rts1b = load(rootb_s1b, bf16)
            rts2a = load(rootb_s2a, bf16); rts2b = load(rootb_s2b, bf16)
            fcawt = {k: {kk: load(vv) for kk, vv in v.items()} for k, v in fcaw.items()}
            src_t = load(srcg, i32)

            Aps = {}
            for et in range(NT):
                for nb in range(3):
                    ap = kp.tile([P, P], f32, name=f"Ap_{et}_{nb}")
                    (nc.gpsimd if AP_GPS else nc.vector).tensor_scalar(
                        out=ap[:], in0=iota_t[:, nb * P:(nb + 1) * P],
                        scalar1=dstl_t[:, et:et + 1], scalar2=None,
                        op0=A.is_equal)
                    Aps[(et, nb)] = ap

            def transpose_into(dst_ap, src_ap, cols, cast=False):
                pt = ps.tile([P, P], f32, name="tp", tag="tp")
                nc.tensor.transpose(out=pt[0:cols, :], in_=src_ap, identity=ident[:])
                if cast:
                    nc.vector.tensor_copy(dst_ap, pt[0:cols, :])
                else:
                    nc.scalar.copy(dst_ap, pt[0:cols, :])

            # ---------- phase 1: a21 + a11 ----------
            agg_s = [psa.tile([P, 512], f32, name=f"agg{nb}", tag=f"agg{nb}")
                     for nb in range(3)]
            for et in range(NT):
                esl = slice(et * P, (et + 1) * P)
                msgs = sb.tile([P, 64], f32, name="msgs")
                for (wname, W2, cin, co, tab) in [
                        ("a21", W2a21, 32, 0, x2o_tab), ("a11", W2a11, 41, 32, x1_tab)]:
                    wt = w_a21 if wname == "a21" else w_a11
                    h1p = ps.tile([32, P], f32, name="h1p", tag="tp")
                    nc.tensor.matmul(h1p[0:32, :], lhsT=wt[:], rhs=eaT_t[:, esl],
                                     start=True, stop=True)
                    h1T = sb.tile([33, P], f32, name="h1T_" + wname)
                    nc.scalar.activation(h1T[0:32, :], h1p[0:32, :], AF.Relu)
                    nc.vector.memset(h1T[32:33, :], 1.0)
                    xs = sb.tile([P, 48], f32, name="xs_s")
                    nc.gpsimd.indirect_dma_start(
                        out=xs[:, 0:cin], out_offset=None, in_=tab[:, :],
                        in_offset=bass.IndirectOffsetOnAxis(ap=src_t[:, et:et + 1], axis=0))
                    oc = 512 // cin
                    for o0 in range(0, 32, oc):
                        no = min(oc, 32 - o0)
                        wid = no * cin
                        wps = pw.tile([P, 512], f32, name="wps", tag="wps")
                        nc.tensor.matmul(wps[:, 0:wid], lhsT=h1T[:],
                                         rhs=W2[:, o0 * cin:(o0 + no) * cin],
                                         start=True, stop=True)
                        tmp = sb.tile([P, 512], bf16, name="tmp_e")
                        nc.vector.tensor_tensor(
                            out=tmp[:, 0:wid].rearrange("p (o i) -> p o i", i=cin),
                            in0=wps[:, 0:wid].rearrange("p (o i) -> p o i", i=cin),
                            in1=bcast(xs[:, 0:cin], no, cin), op=A.mult)
                        nc.vector.tensor_reduce(
                            out=msgs[:, co + o0:co + o0 + no],
                            in_=tmp[:, 0:wid].rearrange("p (o i) -> p o i", i=cin),
                            axis=X, op=A.add)
                for nb in range(3):
                    nc.tensor.matmul(agg_s[nb][:, 0:64], lhsT=Aps[(et, nb)][:],
                                     rhs=msgs[:], start=(et == 0), stop=(et == NT - 1))

            x1v = {}; x2v = {}; x1vT = {}; x2vT = {}
            for nb in range(3):
                nsl = slice(nb * P, (nb + 1) * P)
                nc.tensor.matmul(agg_s[nb][:, 64:96], lhsT=x2oT_t[:, nsl], rhs=rb21[:],
                                 start=True, stop=True)
                nc.tensor.matmul(agg_s[nb][:, 96:128], lhsT=x1T_t[:, nsl], rhs=rb11[:],
                                 start=True, stop=True)
                for (c0, r0, dd) in [(0, 64, x2v), (32, 96, x1v)]:
                    sA = sb.tile([P, 32], f32, name="sA")
                    nc.vector.tensor_scalar(out=sA[:], in0=agg_s[nb][:, c0:c0 + 32],
                                            scalar1=rdeg_t[:, nb:nb + 1], scalar2=None,
                                            op0=A.mult)
                    vv = kp.tile([P, 32], f32, name=f"v{r0}_{nb}")
                    nc.vector.tensor_tensor(out=vv[:], in0=sA[:],
                                            in1=agg_s[nb][:, r0:r0 + 32], op=A.add)
                    nc.vector.tensor_scalar(out=vv[:], in0=vv[:], scalar1=0.0,
                                            scalar2=None, op0=A.max)
                    dd[nb] = vv
                for (dd, dt_, pref) in [(x2v, x2vT, "t2v"), (x1v, x1vT, "t1v")]:
                    tt = kp.tile([33, P], f32, name=f"{pref}_{nb}")
                    transpose_into(tt[0:32, :], dd[nb][:], 32)
                    nc.vector.memset(tt[32:33, :], 1.0)
                    dt_[nb] = tt

            # ---------- phase 2: FCA chain ----------
            def fca_tile(qT_ap, kvT_ap, w, nb, name):
                qkv = psa.tile([P, 96], f32, name=f"qkv_{name}_{nb}", tag=f"agg{nb}")
                nc.tensor.matmul(qkv[:, 0:32], lhsT=qT_ap, rhs=w['wq'][:], start=True, stop=True)
                nc.tensor.matmul(qkv[:, 32:64], lhsT=kvT_ap, rhs=w['wk'][:], start=True, stop=True)
                nc.tensor.matmul(qkv[:, 64:96], lhsT=kvT_ap, rhs=w['wv'][:], start=True, stop=True)
                q_s = sb.tile([P, 32], f32, name="q_s")
                k_s = sb.tile([P, 32], f32, name="k_s")
                v_s = sb.tile([P, 32], bf16, name="v_s")
                nc.scalar.copy(q_s[:], qkv[:, 0:32])
                nc.scalar.copy(k_s[:], qkv[:, 32:64])
                nc.vector.tensor_copy(v_s[:], qkv[:, 64:96])
                lg = sb.tile([P, 1024], f32, name="lg")
                qb = bass.AP(q_s[:].tensor, q_s[:].offset,
                             [q_s[:].ap[0], [1, 32], [0, 32]])
                (nc.gpsimd if FCA_GPS else nc.vector).tensor_tensor(
                    out=lg[:].rearrange("p (i j) -> p i j", j=32),
                    in0=qb, in1=bcast(k_s[:, 0:32], 32, 32), op=A.mult)
                ee = sb.tile([P, 1024], bf16, name="ee")
                nc.scalar.activation(ee[:], lg[:], AF.Exp)
                ev = sb.tile([P, 1024], bf16, name="ev")
                (nc.gpsimd if FCA_GPS else nc.vector).tensor_tensor(
                    out=ev[:].rearrange("p (i j) -> p i j", j=32),
                    in0=ee[:].rearrange("p (i j) -> p i j", j=32),
                    in1=bcast(v_s[:, 0:32], 32, 32), op=A.mult)
                o_u = sb.tile([P, 32], f32, name="o_u")
                s_u = sb.tile([P, 32], f32, name="s_u")
                nc.vector.tensor_reduce(out=o_u[:], in_=ev[:].rearrange("p (i j) -> p i j", j=32),
                                        axis=X, op=A.add)
                nc.vector.tensor_reduce(out=s_u[:], in_=ee[:].rearrange("p (i j) -> p i j", j=32),
                                        axis=X, op=A.add)
                rs = sb.tile([P, 32], f32, name="rs")
                nc.vector.reciprocal(rs[:], s_u[:])
                oo = sb.tile([P, 32], f32, name="oo")
                nc.vector.tensor_tensor(out=oo[:], in0=o_u[:], in1=rs[:], op=A.mult)
                ooT = sb.tile([33, P], f32, name="ooT")
                transpose_into(ooT[0:32, :], oo[:], 32)
                nc.vector.memset(ooT[32:33, :], 1.0)
                op_ = pw.tile([P, 512], f32, name="wps", tag="wps")
                nc.tensor.matmul(op_[:, 0:32], lhsT=ooT[:], rhs=w['wo'][:], start=True, stop=True)
                z = sb.tile([P, 32], f32, name="z_ln", tag=f"z_ln{nb}")
                nc.vector.tensor_tensor(out=z[:], in0=q_s[:], in1=op_[:, 0:32], op=A.add)
                return z

            def ln_finish(zs, name):
                zcs = {}; vss = {}; rsqs = {}
                for nb, z in zs.items():
                    mu = sb.tile([P, 1], f32, name="mu")
                    nc.vector.tensor_reduce(out=mu[:], in_=z[:], axis=X, op=A.add)
                    nc.vector.tensor_scalar(out=mu[:], in0=mu[:], scalar1=1.0 / 32,
                                            scalar2=None, op0=A.mult)
                    zc = sb.tile([P, 32], f32, name="zc", tag=f"zc{nb}")
                    nc.vector.tensor_scalar(out=zc[:], in0=z[:], scalar1=mu[:, 0:1],
                                            scalar2=None, op0=A.subtract)
                    junk = sb.tile([P, 32], bf16, name="junk")
                    vs = sb.tile([P, 1], f32, name="vs", tag=f"vs{nb}")
                    nc.scalar.activation(junk[:], zc[:], AF.Square, accum_out=vs[:])
                    zcs[nb] = zc; vss[nb] = vs
                for nb in zs:
                    lnv = sb.tile([P, 1], f32, name="lnv", tag=f"lnv{nb}")
                    nc.scalar.activation(lnv[:], vss[nb][:], AF.Ln, scale=1.0 / 32,
                                         bias=eps_t[:, 0:1])
                    rsqs[nb] = lnv
                for nb in zs:
                    rsq = sb.tile([P, 1], f32, name="rsq", tag=f"rsq{nb}")
                    nc.scalar.activation(rsq[:], rsqs[nb][:], AF.Exp, scale=-0.5)
                    rsqs[nb] = rsq
                outs = {}
                for nb in zs:
                    out = kp.tile([P, 32], f32, name=f"fca_{name}_{nb}")
                    nc.vector.tensor_scalar(out=out[:], in0=zcs[nb][:],
                                            scalar1=rsqs[nb][:, 0:1],
                                            scalar2=None, op0=A.mult)
                    outT = kp.tile([33, P], f32, name=f"fcaT_{name}_{nb}")
                    transpose_into(outT[0:32, :], out[:], 32)
                    nc.vector.memset(outT[32:33, :], 1.0)
                    outs[nb] = (out, outT)
                return outs

            xcT_a = {}; xcT_b = {}
            zs = {nb: fca_tile(gT_t[:, nb * P:(nb + 1) * P],
                               gT_t[:, nb * P:(nb + 1) * P], fcawt['inter'], nb, "in")
                  for nb in range(3)}
            interR = ln_finish(zs, "in")
            zs = {nb: fca_tile(x3T_t[:, nb * P:(nb + 1) * P], interR[nb][1][:],
                               fcawt['agg'], nb, "gu") for nb in range(3)}
            guR = ln_finish(zs, "gu")
            zs = {nb: fca_tile(x1vT[nb][:], guR[nb][1][:], fcawt['aga'], nb, "gr")
                  for nb in range(3)}
            grpR = ln_finish(zs, "gr")
            zs = {nb: fca_tile(x2vT[nb][:], grpR[nb][1][:], fcawt['aae'], nb, "au")
                  for nb in range(3)}
            auR = ln_finish(zs, "au")
            for nb in range(3):
                nsl = slice(nb * P, (nb + 1) * P)
                inter_o, interT = interR[nb]
                gu_o, guT = guR[nb]
                grp_o, grpT = grpR[nb]
                au_o, auT = auR[nb]
                nc.sync.dma_start(out=intf_out[nsl, :], in_=inter_o[:])
                catT = sb.tile([97, P], f32, name="catT")
                (nc.gpsimd if CP_GPS else nc.vector).tensor_copy(catT[0:32, :], guT[0:32, :])
                (nc.gpsimd if CP_GPS else nc.vector).tensor_copy(catT[32:64, :], grpT[0:32, :])
                (nc.gpsimd if CP_GPS else nc.vector).tensor_copy(catT[64:96, :], auT[0:32, :])
                nc.vector.memset(catT[96:97, :], 1.0)
                xxp = pw.tile([P, 512], f32, name="wps", tag="wps")
                nc.tensor.matmul(xxp[:, 0:96], lhsT=catT[:], rhs=transw_t[:],
                                 start=True, stop=True)
                xx = sb.tile([P, 96], f32, name="xx")
                nc.scalar.activation(xx[:], xxp[:, 0:96], AF.Relu)
                xcat_a = sb.tile([P, P], bf16, name="xcat_a")
                xcat_b = sb.tile([33, P], bf16, name="xcat_b")
                (nc.gpsimd if CP_GPS else nc.vector).tensor_copy(xcat_a[0:32, :], x1vT[nb][0:32, :])
                (nc.gpsimd if CP_GPS else nc.vector).tensor_copy(xcat_a[32:64, :], x2vT[nb][0:32, :])
                ptx = ps.tile([P, P], f32, name="tp", tag="tp")
                nc.tensor.transpose(out=ptx[0:96, :], in_=xx[:], identity=ident[:])
                nc.vector.tensor_copy(xcat_a[64:128, :], ptx[0:64, :])
                nc.vector.tensor_copy(xcat_b[0:32, :], ptx[64:96, :])
                nc.vector.memset(xcat_b[32:33, :], 1.0)
                xcp = pw.tile([P, 512], f32, name="wps", tag="wps")
                nc.tensor.matmul(xcp[:, 0:160], lhsT=xcat_a[:], rhs=xmw_ta[:],
                                 start=True, stop=False)
                nc.tensor.matmul(xcp[:, 0:160], lhsT=xcat_b[:], rhs=xmw_tb[:],
                                 start=False, stop=True)
                xc = sb.tile([P, HID], f32, name="xc")
                nc.scalar.activation(xc[:], xcp[:, 0:160], AF.Relu)
                xa = kp.tile([P, P], bf16, name=f"xcTa{nb}")
                xb = kp.tile([33, P], bf16, name=f"xcTb{nb}")
                transpose_into(xa[:], xc[:, 0:128], 128, cast=True)
                transpose_into(xb[0:32, :], xc[:, 128:160], 32, cast=True)
                nc.vector.memset(xb[32:33, :], 1.0)
                xcT_a[nb] = xa; xcT_b[nb] = xb
                nc.sync.dma_start(out=dram_xc0[nsl, :], in_=xc[:])

            if sim:
                nc.sync.dma_start(out=xc_tab[0:S, :], in_=dram_xc0[:, :])
            else:
                nc.gpsimd.collective_compute(
                    "AllGather", A.bypass, replica_groups=[list(range(NC))],
                    ins=[dram_xc0[:, :]], outs=[xc_tab[:, :]])

            # ---------- big convs ----------
            def bigconv(tab, w_e, rs_a, rs_b, rtwa, rtwb, xTa, xTb, stage):
                agg = [psa.tile([P, 512], f32, name=f"agg{nb}_{stage}", tag=f"agg{nb}")
                       for nb in range(3)]
                msg2s = []
                for et in range(NT):
                    esl = slice(et * P, (et + 1) * P)
                    h1p = ps.tile([P, 96], f32, name="h1pe", tag="tp")
                    nc.tensor.matmul(h1p[:, 0:32], lhsT=eaT_t[:, esl], rhs=w_e[:],
                                     start=True, stop=True)
                    h1 = sb.tile([P, 33], f32, name="h1e")
                    nc.scalar.activation(h1[:, 0:32], h1p[:, 0:32], AF.Relu)
                    nc.vector.memset(h1[:, 32:33], 1.0)
                    xs = sb.tile([P, HID], f32, name="xs_b")
                    nc.gpsimd.indirect_dma_start(
                        out=xs[:], out_offset=None, in_=tab[:, :],
                        in_offset=bass.IndirectOffsetOnAxis(ap=src_t[:, et:et + 1], axis=0))
                    xsTa = sb.tile([P, P], bf16, name="xsTa")
                    xsTb = sb.tile([32, P], bf16, name="xsTb")
                    transpose_into(xsTa[:], xs[:, 0:128], 128, cast=True)
                    transpose_into(xsTb[0:32, :], xs[:, 128:160], 32, cast=True)
                    msg = sb.tile([P, HID], f32, name="msg_b")
                    msgp = pm.tile([P, HID], f32, name="msgp", tag="msgp")
                    nacts = 0
                    for kc in range(11):
                        c0 = kc * 480
                        tps = pw.tile([P, 512], f32, name="wps", tag="wps")
                        nc.tensor.matmul(tps[:, 0:480], lhsT=xsTa[:],
                                         rhs=rs_a[:, c0:c0 + 480], start=True, stop=False)
                        nc.tensor.matmul(tps[:, 0:480], lhsT=xsTb[:],
                                         rhs=rs_b[:, c0:c0 + 480], start=False, stop=True)
                        for j in range(3):
                            k = kc * 3 + j
                            if k == 0:
                                nc.vector.tensor_scalar(
                                    out=msg[:], in0=tps[:, 0:160],
                                    scalar1=h1[:, 0:1], scalar2=None, op0=A.mult)
                            elif ACT_PRED(k):
                                tmpk = sb.tile([P, HID], bf16, name="tmpk")
                                nc.scalar.activation(tmpk[:], tps[:, j * 160:(j + 1) * 160],
                                                     AF.Copy, scale=h1[:, k:k + 1])
                                nc.tensor.matmul(msgp[:], lhsT=identb[:], rhs=tmpk[:],
                                                 start=(nacts == 0), stop=False)
                                nacts += 1
                            else:
                                nc.vector.scalar_tensor_tensor(
                                    out=msg[:], in0=tps[:, j * 160:(j + 1) * 160],
                                    scalar=h1[:, k:k + 1], in1=msg[:],
                                    op0=A.mult, op1=A.add)
                    msg2 = kp.tile([P, HID], f32, name=f"msg2_{stage}_{et}")
                    nc.vector.tensor_tensor(out=msg2[:], in0=msgp[:], in1=msg[:], op=A.add)
                    msg2s.append(msg2)
                for et in range(NT):
                    for nb in range(3):
                        nc.tensor.matmul(agg[nb][:, 0:160], lhsT=Aps[(et, nb)][:],
                                         rhs=msg2s[et][:], start=(et == 0), stop=(et == NT - 1))
                outs = []
                for nb in range(3):
                    nc.tensor.matmul(agg[nb][:, 160:320], lhsT=xTa[nb][:],
                                     rhs=rtwa[:], start=True, stop=False)
                    nc.tensor.matmul(agg[nb][:, 160:320], lhsT=xTb[nb][:],
                                     rhs=rtwb[:], start=False, stop=True)
                    sA = sb.tile([P, HID], f32, name="sAb")
                    nc.vector.tensor_scalar(out=sA[:], in0=agg[nb][:, 0:160],
                                            scalar1=rdeg_t[:, nb:nb + 1], scalar2=None,
                                            op0=A.mult)
                    oo = sb.tile([P, HID], f32, name="oo_b")
                    nc.vector.tensor_tensor(out=oo[:], in0=sA[:],
                                            in1=agg[nb][:, 160:320], op=A.add)
                    o = kp.tile([P, HID], f32, name=f"xcs_{stage}_{nb}")
                    nc.scalar.activation(o[:], oo[:], AF.Relu)
                    outs.append(o)
                return outs

            xc1o = bigconv(xc_tab, w_s1, rs1a, rs1b, rts1a, rts1b, xcT_a, xcT_b, "s1")
            xc1T_a = {}; xc1T_b = {}
            for nb in range(3):
                nc.sync.dma_start(out=dram_xc1[nb * P:(nb + 1) * P, :], in_=xc1o[nb][:])
                xa = kp.tile([P, P], bf16, name=f"x1Ta{nb}")
                xb = kp.tile([33, P], bf16, name=f"x1Tb{nb}")
                transpose_into(xa[:], xc1o[nb][:, 0:128], 128, cast=True)
                transpose_into(xb[0:32, :], xc1o[nb][:, 128:160], 32, cast=True)
                nc.vector.memset(xb[32:33, :], 1.0)
                xc1T_a[nb] = xa; xc1T_b[nb] = xb
            if sim:
                nc.sync.dma_start(out=xc1_tab[0:S, :], in_=dram_xc1[:, :])
            else:
                nc.gpsimd.collective_compute(
                    "AllGather", A.bypass, replica_groups=[list(range(NC))],
                    ins=[dram_xc1[:, :]], outs=[xc1_tab[:, :]])

            xc2o = bigconv(xc1_tab, w_s2, rs2a, rs2b, rts2a, rts2b, xc1T_a, xc1T_b, "s2")
            for nb in range(3):
                nc.sync.dma_start(out=xc2_out[nb * P:(nb + 1) * P, :], in_=xc2o[nb][:])

    nc.compile()
    return nc


def _prep(x, edge_index, edge_attr, batch, params):
    import ml_dtypes
    p = params
    src, dst = np.asarray(edge_index[0]), np.asarray(edge_index[1])
    x = np.asarray(x, np.float32)
    ea_np = np.asarray(edge_attr, np.float32)
    x2o = _host_cfc(x, p)

    order = np.argsort(dst, kind='stable')
    ssrc, sdst, sea = src[order], dst[order], ea_np[order]

    shards = []
    maxe = 0
    for s in range(NC):
        lo, hi = s * S, (s + 1) * S
        m = (sdst >= lo) & (sdst < hi)
        shards.append((ssrc[m], (sdst[m] - lo).astype(np.int64), sea[m]))
        maxe = max(maxe, int(m.sum()))
    EP = ((maxe + P - 1) // P) * P

    key = ("nc", EP)
    if key not in _cache:
        _cache[key] = _build(EP)
    nc = _cache[key]

    def pad_rows(a, n, fill=0.0):
        out = np.full((n,) + a.shape[1:], fill, np.float32)
        out[:a.shape[0]] = a
        return out

    xf = pad_rows(x, NP)
    x2of = pad_rows(np.asarray(x2o, np.float32), NP)
    x1f = np.ascontiguousarray(xf[:, :41])

    def aug(w, b):
        return np.concatenate([np.asarray(w, np.float32),
                               np.asarray(b, np.float32)[None, :]], 0)

    def perm_small(pp, cin):
        ew2 = np.asarray(pp['ew2'], np.float32)
        eb2 = np.asarray(pp['eb2'], np.float32)
        W = np.empty((33, 32 * cin), np.float32)
        w3 = ew2.reshape(32, cin, 32)
        W[0:32] = np.transpose(w3, (0, 2, 1)).reshape(32, 32 * cin)
        W[32] = eb2.reshape(cin, 32).T.reshape(-1)
        return W

    def perm_big(pp):
        ew2 = np.asarray(pp['ew2'], np.float32).reshape(32, HID, HID)
        eb2 = np.asarray(pp['eb2'], np.float32).reshape(HID, HID)
        R = np.empty((HID, 33 * HID), np.float32)
        for k in range(32):
            R[:, k * HID:(k + 1) * HID] = ew2[k]
        R[:, 32 * HID:] = eb2
        return R

    fca_in = {}
    for nm in ['inter', 'agg', 'aga', 'aae']:
        w = p[nm]
        fca_in[f"fca_{nm}_wq"] = aug(w['wq'], w['bq'])
        fca_in[f"fca_{nm}_wk"] = aug(w['wk'], w['bk'])
        fca_in[f"fca_{nm}_wv"] = aug(w['wv'], w['bv'])
        fca_in[f"fca_{nm}_wo"] = aug(w['wo'], w['bo'])

    rb_s1 = perm_big(p['sub1']); rb_s2 = perm_big(p['sub2'])
    bf = ml_dtypes.bfloat16

    in_maps = []
    for s in range(NC):
        es, ed, ea = shards[s]
        ne = len(es)
        srcg_f = np.zeros(EP, np.int32); srcg_f[:ne] = es
        srcg = np.ascontiguousarray(srcg_f.reshape(EP // P, P).T)
        dstl_f = np.full(EP, -1000.0, np.float32); dstl_f[:ne] = ed
        dstl = np.ascontiguousarray(dstl_f.reshape(EP // P, P).T)
        eaT_in = np.ones((11, EP), np.float32)
        eaT_in[0:10, :ne] = ea.T
        eaT_in[0:10, ne:] = 0.0
        deg = np.bincount(ed, minlength=S)[:S].astype(np.float32)
        rdeg_in = (1.0 / np.maximum(deg, 1.0)).reshape(3, P).T.copy()
        lo = s * S
        xo = xf[lo:lo + S]
        ones_row = np.ones((1, S), np.float32)
        m = dict(
            x2o_tab=x2of, x1_tab=x1f, eaT=eaT_in, srcg=srcg, dstl=dstl,
            iota=np.broadcast_to(np.arange(S, dtype=np.float32)[None, :], (P, S)).copy(),
            rdeg=rdeg_in,
            gT=np.concatenate([xo[:, 67:86].T, ones_row], 0),
            x3T=np.concatenate([xo[:, 48:67].T, ones_row], 0),
            x1T=np.concatenate([xo[:, 0:41].T, ones_row], 0),
            x2oT=np.concatenate([x2of[lo:lo + S].T, ones_row], 0),
            ew1b_a21=aug(p['a21']['ew1'], p['a21']['eb1']),
            ew1b_a11=aug(p['a11']['ew1'], p['a11']['eb1']),
            W2p_a21=perm_small(p['a21'], 32), W2p_a11=perm_small(p['a11'], 41),
            rootb_a21=aug(p['a21']['root'], p['a21']['bias']),
            rootb_a11=aug(p['a11']['root'], p['a11']['bias']),
            transw=aug(p['trans_w'], p['trans_b']),
            xmw_a=aug(p['xm_w'], p['xm_b'])[0:128].astype(bf),
            xmw_b=aug(p['xm_w'], p['xm_b'])[128:161].astype(bf),
            ew1b_s1=aug(p['sub1']['ew1'], p['sub1']['eb1']),
            ew1b_s2=aug(p['sub2']['ew1'], p['sub2']['eb1']),
            rhsb_s1a=rb_s1[0:128].astype(bf), rhsb_s1b=rb_s1[128:160].astype(bf),
            rhsb_s2a=rb_s2[0:128].astype(bf), rhsb_s2b=rb_s2[128:160].astype(bf),
            rootb_s1a=aug(p['sub1']['root'], p['sub1']['bias'])[0:128].astype(bf),
            rootb_s1b=aug(p['sub1']['root'], p['sub1']['bias'])[128:161].astype(bf),
            rootb_s2a=aug(p['sub2']['root'], p['sub2']['bias'])[0:128].astype(bf),
            rootb_s2b=aug(p['sub2']['root'], p['sub2']['bias'])[128:161].astype(bf),
            **{k: v.copy() for k, v in fca_in.items()},
        )
        in_maps.append(m)
    return nc, in_maps, ea_np


def kernel(x, edge_index, edge_attr, batch, params):
    import time
    from concourse.bass_utils import run_bass_kernel_spmd
    nc, in_maps, ea_np = _prep(x, edge_index, edge_attr, batch, params)
    t0 = time.time()
    res = run_bass_kernel_spmd(nc, in_maps, core_ids=list(range(NC)))
    dt_ns = int((time.time() - t0) * 1e9)
    _cache['exec_ns'] = min(_cache.get('exec_ns', 1 << 62), dt_ns)
    xc2 = np.concatenate([res.results[s]["xc2"] for s in range(NC)], 0)[:N]
    intf = np.concatenate([res.results[s]["interf"] for s in range(NC)], 0)[:N]
    src, dst = np.asarray(edge_index[0]), np.asarray(edge_index[1])
    y = _afp_tail(xc2, intf, src, dst, np.asarray(batch), ea_np, params)
    return y.astype(np.float32)


# revision 17
# speedup vs baseline: 1.0411x; 1.0140x over previous
"""MesoNet Trainium2 kernel: 8-core SPMD.

Device computes the dominant stages (NNConv a21/a11, 4x FCA+LN, trans/xm,
sub1/sub2 big edge-conditioned convs) with nodes + their incoming edges
sharded per core and AllGathers between conv layers.  The small graph-level
AttentiveFP tail (~2% of FLOPs) runs on host.
"""
import numpy as np

N, E, G = 3000, 6000, 150
NP = 3072
S = 384
NC = 8
HID = 160
P = 128

_cache = {}
ACT_PRED = lambda k: k % 2 == 1 and k < 31
SB_BUFS = 3
FCA_GPS = False
AP_GPS = False
CP_GPS = False


# ---------------- host math ----------------
def _sig(v):
    return 1.0 / (1.0 + np.exp(-v))


def _lrelu(v, a=0.01):
    return np.where(v >= 0, v, a * v)


def _elu(v):
    return np.where(v >= 0, v, np.expm1(v))


def _relu(v):
    return np.maximum(v, 0.0)


def _seg_sum(v, seg, n):
    out = np.zeros((n,) + v.shape[1:], np.float32)
    np.add.at(out, seg, v)
    return out


def _seg_softmax(a, seg, n):
    e = np.exp(a)
    s = _seg_sum(e, seg, n)
    return e / np.maximum(s[seg], 1e-16)


def _gru(xv, h, q):
    gi = xv @ q['wih'] + q['bih']
    gh = h @ q['whh'] + q['bhh']
    ir, iz, inn = np.split(gi, 3, -1)
    hr, hz, hn = np.split(gh, 3, -1)
    r = _sig(ir + hr)
    zt = _sig(iz + hz)
    nn_ = np.tanh(inn + r * hn)
    return (1 - zt) * nn_ + zt * h


def _gat(x_src, x_dst, src, dst, q, n_dst):
    hs = x_src @ q['w']
    hd = x_dst @ q['w']
    a = _lrelu((hs @ q['att_src'])[src] + (hd @ q['att_dst'])[dst])
    alpha = _seg_softmax(a, dst, n_dst)
    return _seg_sum(alpha[:, None] * hs[src], dst, n_dst) + q['bias']


def _afp_tail(xc, inter_f, src, dst, batch, edge_attr, p):
    q = p['afp']
    xv = _lrelu(xc @ q['lin1_w'] + q['lin1_b'])
    xj = _lrelu(np.concatenate([xv[src], edge_attr], -1) @ q['gate_lin1'])
    a = _lrelu(xj @ q['gate_att_l'] + (xv @ q['gate_att_r'])[dst])
    alpha = _seg_softmax(a, dst, N)
    hh = _seg_sum((xv[src] @ q['gate_lin2']) * alpha[:, None], dst, N) + q['gate_bias']
    xv = _relu(_gru(_elu(hh), xv, q['gru1']))
    hh = _elu(_gat(xv, xv, src, dst, q['conv1'], N))
    xv = _relu(_gru(hh, xv, q['gru2']))
    out = _relu(_seg_sum(xv, batch, G))
    row = np.arange(N)
    for _ in range(2):
        hh = _elu(_gat(xv, out, row, batch, q['mol_conv'], G))
        out = _relu(_gru(hh, out, q['mol_gru']))
    xg = out @ q['lin2_w'] + q['lin2_b']
    cnt = _seg_sum(np.ones(N, np.float32), batch, G)
    grp_pool = _relu((_seg_sum(inter_f, batch, G) / np.maximum(cnt, 1.0)[:, None])
                     @ p['group_w'] + p['group_b'])
    y = np.concatenate([xg, grp_pool], -1)
    y = _relu(y @ p['fc1_w'] + p['fc1_b'])
    y = _relu(y @ p['fc2_w'] + p['fc2_b'])
    y = _relu(y @ p['fc3_w'] + p['fc3_b'])
    return y @ p['fc4_w'] + p['fc4_b']


def _host_cfc(x, p):
    x2 = x[:, 42:48]
    lt = lambda v: 1.7159 * np.tanh(0.666 * v)
    c = p['cfc']
    h = np.concatenate([x2, x2], -1)
    outs = []
    for _ in range(5):
        z = np.concatenate([x2, h], -1)
        ti = _sig(z @ c['wta'] + c['bta'] + z @ c['wtb'] + c['btb'])
        h = lt(z @ c['wf1'] + c['bf1']) * (1.0 - ti) + ti * lt(z @ c['wf2'] + c['bf2'])
        outs.append(h[:, :6])
    return _relu(np.concatenate(outs, -1) @ p['x22_w'] + p['x22_b'])


# ---------------- device kernel ----------------
def _build(EP, sim=False):
    import concourse.bass as bass
    import concourse.mybir as mybir
    import concourse.tile as tile
    from concourse import bacc
    from concourse.masks import make_identity

    f32 = mybir.dt.float32
    bf16 = mybir.dt.bfloat16
    i32 = mybir.dt.int32
    A = mybir.AluOpType
    AF = mybir.ActivationFunctionType
    X = mybir.AxisListType.X
    NT = EP // P

    def bcast(ap, rep, axis_len):
        """[P, axis_len] -> [P, rep, axis_len] with stride-0 middle dim."""
        return bass.AP(ap.tensor, ap.offset, [ap.ap[0], [0, rep], ap.ap[1]])

    nc = bacc.Bacc("TRN2", target_bir_lowering=False, debug=False, num_devices=NC)

    def din(name, shape, dt=f32):
        return nc.dram_tensor(name, shape, dt, kind="ExternalInput")

    x2o_tab = din("x2o_tab", [NP, 32])
    x1_tab = din("x1_tab", [NP, 41])
    eaT = din("eaT", [11, EP])
    srcg = din("srcg", [P, EP // P], i32)
    dstl = din("dstl", [P, EP // P])
    iota = din("iota", [P, S])
    rdeg = din("rdeg", [P, 3])
    gT = din("gT", [20, S]); x3T = din("x3T", [20, S])
    x1T = din("x1T", [42, S]); x2oT = din("x2oT", [33, S])
    ew1b_a21 = din("ew1b_a21", [11, 32]); ew1b_a11 = din("ew1b_a11", [11, 32])
    W2p_a21 = din("W2p_a21", [33, 1024]); W2p_a11 = din("W2p_a11", [33, 1312])
    rootb_a21 = din("rootb_a21", [33, 32]); rootb_a11 = din("rootb_a11", [42, 32])
    fcaw = {}
    for nm, dq, dkv in [("inter", 19, 19), ("agg", 19, 32), ("aga", 32, 32), ("aae", 32, 32)]:
        fcaw[nm] = dict(
            wq=din(f"fca_{nm}_wq", [dq + 1, 32]), wk=din(f"fca_{nm}_wk", [dkv + 1, 32]),
            wv=din(f"fca_{nm}_wv", [dkv + 1, 32]), wo=din(f"fca_{nm}_wo", [33, 32]))
    transw = din("transw", [97, 96])
    xmw_a = din("xmw_a", [128, 160], bf16)
    xmw_b = din("xmw_b", [33, 160], bf16)
    ew1b_s1 = din("ew1b_s1", [11, 32]); ew1b_s2 = din("ew1b_s2", [11, 32])
    rhsb_s1a = din("rhsb_s1a", [128, 5280], bf16); rhsb_s1b = din("rhsb_s1b", [32, 5280], bf16)
    rhsb_s2a = din("rhsb_s2a", [128, 5280], bf16); rhsb_s2b = din("rhsb_s2b", [32, 5280], bf16)
    rootb_s1a = din("rootb_s1a", [128, 160], bf16); rootb_s1b = din("rootb_s1b", [33, 160], bf16)
    rootb_s2a = din("rootb_s2a", [128, 160], bf16); rootb_s2b = din("rootb_s2b", [33, 160], bf16)

    xc2_out = nc.dram_tensor("xc2", [S, HID], f32, kind="ExternalOutput")
    intf_out = nc.dram_tensor("interf", [S, 32], f32, kind="ExternalOutput")

    with tile.TileContext(nc) as tc:
        with (
            tc.tile_pool(name="sb", bufs=SB_BUFS) as sb,
            tc.tile_pool(name="keep", bufs=1) as kp,
            tc.tile_pool(name="ps", bufs=2, space="PSUM") as ps,
            tc.tile_pool(name="pw", bufs=2, space="PSUM") as pw,
            tc.tile_pool(name="pm", bufs=1, space="PSUM") as pm,
            tc.tile_pool(name="psagg", bufs=1, space="PSUM") as psa,
            tc.tile_pool(name="dram", bufs=1, space="DRAM") as dram,
        ):
            dram_xc0 = dram.tile([S, HID], f32, name="dram_xc0")
            xc_tab = dram.tile([NP, HID], f32, name="xc_tab", addr_space="Shared")
            dram_xc1 = dram.tile([S, HID], f32, name="dram_xc1")
            xc1_tab = dram.tile([NP, HID], f32, name="xc1_tab", addr_space="Shared")

            ident = kp.tile([P, P], f32, name="ident")
            make_identity(nc, ident[:])
            eps_t = kp.tile([P, 1], f32, name="eps_t")
            nc.vector.memset(eps_t[:], 1e-5)
            identb = kp.tile([P, P], bf16, name="identb")
            nc.vector.tensor_copy(identb[:], ident[:])

            def load(t, dt=f32):
                tl = kp.tile(list(t.shape), dt, name="ld_" + t.name)
                nc.sync.dma_start(out=tl[:], in_=t[:, :])
                return tl

            iota_t = load(iota); dstl_t = load(dstl); rdeg_t = load(rdeg)
            eaT_t = load(eaT)
            gT_t = load(gT); x3T_t = load(x3T); x1T_t = load(x1T); x2oT_t = load(x2oT)
            w_a21 = load(ew1b_a21); w_a11 = load(ew1b_a11)
            W2a21 = load(W2p_a21); W2a11 = load(W2p_a11)
            rb21 = load(rootb_a21); rb11 = load(rootb_a11)
            transw_t = load(transw)
            xmw_ta = load(xmw_a, bf16); xmw_tb = load(xmw_b, bf16)
            w_s1 = load(ew1b_s1); w_s2 = load(ew1b_s2)
            rs1a = load(rhsb_s1a, bf16); rs1b = load(rhsb_s1b, bf16)
            rs2a = load(rhsb_s2a, bf16); rs2b = load(rhsb_s2b, bf16)
            rts1a = load(rootb_s1a, bf16); rts1b = load(rootb_s1b, bf16)
            rts2a = load(rootb_s2a, bf16); rts2b = load(rootb_s2b, bf16)
            fcawt = {k: {kk: load(vv) for kk, vv in v.items()} for k, v in fcaw.items()}
            src_t = load(srcg, i32)

            Aps = {}
            for et in range(NT):
                for nb in range(3):
                    ap = kp.tile([P, P], f32, name=f"Ap_{et}_{nb}")
                    (nc.gpsimd if AP_GPS else nc.vector).tensor_scalar(
                        out=ap[:], in0=iota_t[:, nb * P:(nb + 1) * P],
                        scalar1=dstl_t[:, et:et + 1], scalar2=None,
                        op0=A.is_equal)
                    Aps[(et, nb)] = ap

            def transpose_into(dst_ap, src_ap, cols, cast=False):
                pt = ps.tile([P, P], f32, name="tp", tag="tp")
                nc.tensor.transpose(out=pt[0:cols, :], in_=src_ap, identity=ident[:])
                if cast:
                    nc.vector.tensor_copy(dst_ap, pt[0:cols, :])
                else:
                    nc.scalar.copy(dst_ap, pt[0:cols, :])

            # ---------- phase 1: a21 + a11 ----------
            agg_s = [psa.tile([P, 512], f32, name=f"agg{nb}", tag=f"agg{nb}")
                     for nb in range(3)]
            for et in range(NT):
                esl = slice(et * P, (et + 1) * P)
                msgs = sb.tile([P, 64], f32, name="msgs")
                for (wname, W2, cin, co, tab) in [
                        ("a21", W2a21, 32, 0, x2o_tab), ("a11", W2a11, 41, 32, x1_tab)]:
                    wt = w_a21 if wname == "a21" else w_a11
                    h1p = ps.tile([32, P], f32, name="h1p", tag="tp")
                    nc.tensor.matmul(h1p[0:32, :], lhsT=wt[:], rhs=eaT_t[:, esl],
                                     start=True, stop=True)
                    h1T = sb.tile([33, P], f32, name="h1T_" + wname)
                    nc.scalar.activation(h1T[0:32, :], h1p[0:32, :], AF.Relu)
                    nc.vector.memset(h1T[32:33, :], 1.0)
                    xs = sb.tile([P, 48], f32, name="xs_s")
                    nc.gpsimd.indirect_dma_start(
                        out=xs[:, 0:cin], out_offset=None, in_=tab[:, :],
                        in_offset=bass.IndirectOffsetOnAxis(ap=src_t[:, et:et + 1], axis=0))
                    oc = 512 // cin
                    for o0 in range(0, 32, oc):
                        no = min(oc, 32 - o0)
                        wid = no * cin
                        wps = pw.tile([P, 512], f32, name="wps", tag="wps")
                        nc.tensor.matmul(wps[:, 0:wid], lhsT=h1T[:],
                                         rhs=W2[:, o0 * cin:(o0 + no) * cin],
                                         start=True, stop=True)
                        tmp = sb.tile([P, 512], bf16, name="tmp_e")
                        nc.vector.tensor_tensor(
                            out=tmp[:, 0:wid].rearrange("p (o i) -> p o i", i=cin),
                            in0=wps[:, 0:wid].rearrange("p (o i) -> p o i", i=cin),
                            in1=bcast(xs[:, 0:cin], no, cin), op=A.mult)
                        nc.vector.tensor_reduce(
                            out=msgs[:, co + o0:co + o0 + no],
                            in_=tmp[:, 0:wid].rearrange("p (o i) -> p o i", i=cin),
                            axis=X, op=A.add)
                for nb in range(3):
                    nc.tensor.matmul(agg_s[nb][:, 0:64], lhsT=Aps[(et, nb)][:],
                                     rhs=msgs[:], start=(et == 0), stop=(et == NT - 1))

            x1v = {}; x2v = {}; x1vT = {}; x2vT = {}
            for nb in range(3):
                nsl = slice(nb * P, (nb + 1) * P)
                nc.tensor.matmul(agg_s[nb][:, 64:96], lhsT=x2oT_t[:, nsl], rhs=rb21[:],
                                 start=True, stop=True)
                nc.tensor.matmul(agg_s[nb][:, 96:128], lhsT=x1T_t[:, nsl], rhs=rb11[:],
                                 start=True, stop=True)
                for (c0, r0, dd) in [(0, 64, x2v), (32, 96, x1v)]:
                    sA = sb.tile([P, 32], f32, name="sA")
                    nc.vector.tensor_scalar(out=sA[:], in0=agg_s[nb][:, c0:c0 + 32],
                                            scalar1=rdeg_t[:, nb:nb + 1], scalar2=None,
                                            op0=A.mult)
                    vv = kp.tile([P, 32], f32, name=f"v{r0}_{nb}")
                    nc.vector.tensor_tensor(out=vv[:], in0=sA[:],
                                            in1=agg_s[nb][:, r0:r0 + 32], op=A.add)
                    nc.vector.tensor_scalar(out=vv[:], in0=vv[:], scalar1=0.0,
                                            scalar2=None, op0=A.max)
                    dd[nb] = vv
                for (dd, dt_, pref) in [(x2v, x2vT, "t2v"), (x1v, x1vT, "t1v")]:
                    tt = kp.tile([33, P], f32, name=f"{pref}_{nb}")
                    transpose_into(tt[0:32, :], dd[nb][:], 32)
                    nc.vector.memset(tt[32:33, :], 1.0)
                    dt_[nb] = tt

            # ---------- phase 2: FCA chain ----------
            def fca_tile(qT_ap, kvT_ap, w, nb, name):
                qkv = psa.tile([P, 96], f32, name=f"qkv_{name}_{nb}", tag=f"agg{nb}")
                nc.tensor.matmul(qkv[:, 0:32], lhsT=qT_ap, rhs=w['wq'][:], start=True, stop=True)
                nc.tensor.matmul(qkv[:, 32:64], lhsT=kvT_ap, rhs=w['wk'][:], start=True, stop=True)
                nc.tensor.matmul(qkv[:, 64:96], lhsT=kvT_ap, rhs=w['wv'][:], start=True, stop=True)
                q_s = sb.tile([P, 32], f32, name="q_s")
                k_s = sb.tile([P, 32], f32, name="k_s")
                v_s = sb.tile([P, 32], bf16, name="v_s")
                nc.scalar.copy(q_s[:], qkv[:, 0:32])
                nc.scalar.copy(k_s[:], qkv[:, 32:64])
                nc.vector.tensor_copy(v_s[:], qkv[:, 64:96])
                lg = sb.tile([P, 1024], f32, name="lg")
                qb = bass.AP(q_s[:].tensor, q_s[:].offset,
                             [q_s[:].ap[0], [1, 32], [0, 32]])
                (nc.gpsimd if FCA_GPS else nc.vector).tensor_tensor(
                    out=lg[:].rearrange("p (i j) -> p i j", j=32),
                    in0=qb, in1=bcast(k_s[:, 0:32], 32, 32), op=A.mult)
                ee = sb.tile([P, 1024], bf16, name="ee")
                nc.scalar.activation(ee[:], lg[:], AF.Exp)
                ev = sb.tile([P, 1024], bf16, name="ev")
                (nc.gpsimd if FCA_GPS else nc.vector).tensor_tensor(
                    out=ev[:].rearrange("p (i j) -> p i j", j=32),
                    in0=ee[:].rearrange("p (i j) -> p i j", j=32),
                    in1=bcast(v_s[:, 0:32], 32, 32), op=A.mult)
                o_u = sb.tile([P, 32], f32, name="o_u")
                s_u = sb.tile([P, 32], f32, name="s_u")
                nc.vector.tensor_reduce(out=o_u[:], in_=ev[:].rearrange("p (i j) -> p i j", j=32),
                                        axis=X, op=A.add)
                nc.vector.tensor_reduce(out=s_u[:], in_=ee[:].rearrange("p (i j) -> p i j", j=32),
                                        axis=X, op=A.add)
                rs = sb.tile([P, 32], f32, name="rs")
                nc.vector.reciprocal(rs[:], s_u[:])
                oo = sb.tile([P, 32], f32, name="oo")
                nc.vector.tensor_tensor(out=oo[:], in0=o_u[:], in1=rs[:], op=A.mult)
                ooT = sb.tile([33, P], f32, name="ooT")
                transpose_into(ooT[0:32, :], oo[:], 32)
                nc.vector.memset(ooT[32:33, :], 1.0)
                op_ = pw.tile([P, 512], f32, name="wps", tag="wps")
                nc.tensor.matmul(op_[:, 0:32], lhsT=ooT[:], rhs=w['wo'][:], start=True, stop=True)
                z = sb.tile([P, 32], f32, name="z_ln", tag=f"z_ln{nb}")
                nc.vector.tensor_tensor(out=z[:], in0=q_s[:], in1=op_[:, 0:32], op=A.add)
                return z

            def ln_finish(zs, name):
                zcs = {}; vss = {}; rsqs = {}
                for nb, z in zs.items():
                    mu = sb.tile([P, 1], f32, name="mu")
                    nc.vector.tensor_reduce(out=mu[:], in_=z[:], axis=X, op=A.add)
                    nc.vector.tensor_scalar(out=mu[:], in0=mu[:], scalar1=1.0 / 32,
                                            scalar2=None, op0=A.mult)
                    zc = sb.tile([P, 32], f32, name="zc", tag=f"zc{nb}")
                    nc.vector.tensor_scalar(out=zc[:], in0=z[:], scalar1=mu[:, 0:1],
                                            scalar2=None, op0=A.subtract)
                    junk = sb.tile([P, 32], bf16, name="junk")
                    vs = sb.tile([P, 1], f32, name="vs", tag=f"vs{nb}")
                    nc.scalar.activation(junk[:], zc[:], AF.Square, accum_out=vs[:])
                    zcs[nb] = zc; vss[nb] = vs
                for nb in zs:
                    lnv = sb.tile([P, 1], f32, name="lnv", tag=f"lnv{nb}")
                    nc.scalar.activation(lnv[:], vss[nb][:], AF.Ln, scale=1.0 / 32,
                                         bias=eps_t[:, 0:1])
                    rsqs[nb] = lnv
                for nb in zs:
                    rsq = sb.tile([P, 1], f32, name="rsq", tag=f"rsq{nb}")
                    nc.scalar.activation(rsq[:], rsqs[nb][:], AF.Exp, scale=-0.5)
                    rsqs[nb] = rsq
                outs = {}
                for nb in zs:
                    out = kp.tile([P, 32], f32, name=f"fca_{name}_{nb}")
                    nc.vector.tensor_scalar(out=out[:], in0=zcs[nb][:],
                                            scalar1=rsqs[nb][:, 0:1],
                                            scalar2=None, op0=A.mult)
                    outT = kp.tile([33, P], f32, name=f"fcaT_{name}_{nb}")
                    transpose_into(outT[0:32, :], out[:], 32)
                    nc.vector.memset(outT[32:33, :], 1.0)
                    outs[nb] = (out, outT)
                return outs

            xcT_a = {}; xcT_b = {}
            zs = {nb: fca_tile(gT_t[:, nb * P:(nb + 1) * P],
                               gT_t[:, nb * P:(nb + 1) * P], fcawt['inter'], nb, "in")
                  for nb in range(3)}
            interR = ln_finish(zs, "in")
            zs = {nb: fca_tile(x3T_t[:, nb * P:(nb + 1) * P], interR[nb][1][:],
                               fcawt['agg'], nb, "gu") for nb in range(3)}
            guR = ln_finish(zs, "gu")
            zs = {nb: fca_tile(x1vT[nb][:], guR[nb][1][:], fcawt['aga'], nb, "gr")
                  for nb in range(3)}
            grpR = ln_finish(zs, "gr")
            zs = {nb: fca_tile(x2vT[nb][:], grpR[nb][1][:], fcawt['aae'], nb, "au")
                  for nb in range(3)}
            auR = ln_finish(zs, "au")
            for nb in range(3):
                nsl = slice(nb * P, (nb + 1) * P)
                inter_o, interT = interR[nb]
                gu_o, guT = guR[nb]
                grp_o, grpT = grpR[nb]
                au_o, auT = auR[nb]
                nc.sync.dma_start(out=intf_out[nsl, :], in_=inter_o[:])
                catT = sb.tile([97, P], f32, name="catT")
                (nc.gpsimd if CP_GPS else nc.vector).tensor_copy(catT[0:32, :], guT[0:32, :])
                (nc.gpsimd if CP_GPS else nc.vector).tensor_copy(catT[32:64, :], grpT[0:32, :])
                (nc.gpsimd if CP_GPS else nc.vector).tensor_copy(catT[64:96, :], auT[0:32, :])
                nc.vector.memset(catT[96:97, :], 1.0)
                xxp = pw.tile([P, 512], f32, name="wps", tag="wps")
                nc.tensor.matmul(xxp[:, 0:96], lhsT=catT[:], rhs=transw_t[:],
                                 start=True, stop=True)
                xx = sb.tile([P, 96], f32, name="xx")
                nc.scalar.activation(xx[:], xxp[:, 0:96], AF.Relu)
                xcat_a = sb.tile([P, P], bf16, name="xcat_a")
                xcat_b = sb.tile([33, P], bf16, name="xcat_b")
                (nc.gpsimd if CP_GPS else nc.vector).tensor_copy(xcat_a[0:32, :], x1vT[nb][0:32, :])
                (nc.gpsimd if CP_GPS else nc.vector).tensor_copy(xcat_a[32:64, :], x2vT[nb][0:32, :])
                ptx = ps.tile([P, P], f32, name="tp", tag="tp")
                nc.tensor.transpose(out=ptx[0:96, :], in_=xx[:], identity=ident[:])
                nc.vector.tensor_copy(xcat_a[64:128, :], ptx[0:64, :])
                nc.vector.tensor_copy(xcat_b[0:32, :], ptx[64:96, :])
                nc.vector.memset(xcat_b[32:33, :], 1.0)
                xcp = pw.tile([P, 512], f32, name="wps", tag="wps")
                nc.tensor.matmul(xcp[:, 0:160], lhsT=xcat_a[:], rhs=xmw_ta[:],
                                 start=True, stop=False)
                nc.tensor.matmul(xcp[:, 0:160], lhsT=xcat_b[:], rhs=xmw_tb[:],
                                 start=False, stop=True)
                xc = sb.tile([P, HID], f32, name="xc")
                nc.scalar.activation(xc[:], xcp[:, 0:160], AF.Relu)
                xa = kp.tile([P, P], bf16, name=f"xcTa{nb}")
                xb = kp.tile([33, P], bf16, name=f"xcTb{nb}")
                transpose_into(xa[:], xc[:, 0:128], 128, cast=True)
                transpose_into(xb[0:32, :], xc[:, 128:160], 32, cast=True)
                nc.vector.memset(xb[32:33, :], 1.0)
                xcT_a[nb] = xa; xcT_b[nb] = xb
                nc.sync.dma_start(out=dram_xc0[nsl, :], in_=xc[:])

            if sim:
                nc.sync.dma_start(out=xc_tab[0:S, :], in_=dram_xc0[:, :])
            else:
                nc.gpsimd.collective_compute(
                    "AllGather", A.bypass, replica_groups=[list(range(NC))],
                    ins=[dram_xc0[:, :]], outs=[xc_tab[:, :]])

            # ---------- big convs ----------
            def bigconv(tab, w_e, rs_a, rs_b, rtwa, rtwb, xTa, xTb, stage):
                agg = [psa.tile([P, 512], f32, name=f"agg{nb}_{stage}", tag=f"agg{nb}")
                       for nb in range(3)]
                msg2s = []
                for et in range(NT):
                    esl = slice(et * P, (et + 1) * P)
                    h1p = ps.tile([P, 96], f32, name="h1pe", tag="tp")
                    nc.tensor.matmul(h1p[:, 0:32], lhsT=eaT_t[:, esl], rhs=w_e[:],
                                     start=True, stop=True)
                    h1 = sb.tile([P, 33], f32, name="h1e")
                    nc.scalar.activation(h1[:, 0:32], h1p[:, 0:32], AF.Relu)
                    nc.vector.memset(h1[:, 32:33], 1.0)
                    xs = sb.tile([P, HID], f32, name="xs_b")
                    nc.gpsimd.indirect_dma_start(
                        out=xs[:], out_offset=None, in_=tab[:, :],
                        in_offset=bass.IndirectOffsetOnAxis(ap=src_t[:, et:et + 1], axis=0))
                    xsTa = sb.tile([P, P], bf16, name="xsTa")
                    xsTb = sb.tile([32, P], bf16, name="xsTb")
                    transpose_into(xsTa[:], xs[:, 0:128], 128, cast=True)
                    transpose_into(xsTb[0:32, :], xs[:, 128:160], 32, cast=True)
                    msg = sb.tile([P, HID], f32, name="msg_b")
                    msgp = pm.tile([P, HID], f32, name="msgp", tag="msgp")
                    nacts = 0
                    for kc in range(11):
                        c0 = kc * 480
                        tps = pw.tile([P, 512], f32, name="wps", tag="wps")
                        nc.tensor.matmul(tps[:, 0:480], lhsT=xsTa[:],
                                         rhs=rs_a[:, c0:c0 + 480], start=True, stop=False)
                        nc.tensor.matmul(tps[:, 0:480], lhsT=xsTb[:],
                                         rhs=rs_b[:, c0:c0 + 480], start=False, stop=True)
                        for j in range(3):
                            k = kc * 3 + j
                            if k == 0:
                                nc.vector.tensor_scalar(
                                    out=msg[:], in0=tps[:, 0:160],
                                    scalar1=h1[:, 0:1], scalar2=None, op0=A.mult)
                            elif ACT_PRED(k):
                                tmpk = sb.tile([P, HID], bf16, name="tmpk")
                                nc.scalar.activation(tmpk[:], tps[:, j * 160:(j + 1) * 160],
                                                     AF.Copy, scale=h1[:, k:k + 1])
                                nc.tensor.matmul(msgp[:], lhsT=identb[:], rhs=tmpk[:],
                                                 start=(nacts == 0), stop=False)
                                nacts += 1
                            else:
                                nc.vector.scalar_tensor_tensor(
                                    out=msg[:], in0=tps[:, j * 160:(j + 1) * 160],
                                    scalar=h1[:, k:k + 1], in1=msg[:],
                                    op0=A.mult, op1=A.add)
                    msg2 = kp.tile([P, HID], f32, name=f"msg2_{stage}_{et}")
                    nc.vector.tensor_tensor(out=msg2[:], in0=msgp[:], in1=msg[:], op=A.add)
                    msg2s.append(msg2)
                for et in range(NT):
                    for nb in range(3):
                        nc.tensor.matmul(agg[nb][:, 0:160], lhsT=Aps[(et, nb)][:],
                                         rhs=msg2s[et][:], start=(et == 0), stop=(et == NT - 1))
                outs = []
                for nb in range(3):
                    nc.tensor.matmul(agg[nb][:, 160:320], lhsT=xTa[nb][:],
                                     rhs=rtwa[:], start=True, stop=False)
                    nc.tensor.matmul(agg[nb][:, 160:320], lhsT=xTb[nb][:],
                                     rhs=rtwb[:], start=False, stop=True)
                    sA = sb.tile([P, HID], f32, name="sAb")
                    nc.vector.tensor_scalar(out=sA[:], in0=agg[nb][:, 0:160],
                                            scalar1=rdeg_t[:, nb:nb + 1], scalar2=None,
                                            op0=A.mult)
                    oo = sb.tile([P, HID], f32, name="oo_b")
                    nc.vector.tensor_tensor(out=oo[:], in0=sA[:],
                                            in1=agg[nb][:, 160:320], op=A.add)
                    o = kp.tile([P, HID], f32, name=f"xcs_{stage}_{nb}")
                    nc.scalar.activation(o[:], oo[:], AF.Relu)
                    outs.append(o)
                return outs

            xc1o = bigconv(xc_tab, w_s1, rs1a, rs1b, rts1a, rts1b, xcT_a, xcT_b, "s1")
            xc1T_a = {}; xc1T_b = {}
            for nb in range(3):
                nc.sync.dma_start(out=dram_xc1[nb * P:(nb + 1) * P, :], in_=xc1o[nb][:])
                xa = kp.tile([P, P], bf16, name=f"x1Ta{nb}")
                xb = kp.tile([33, P], bf16, name=f"x1Tb{nb}")
                transpose_into(xa[:], xc1o[nb][:, 0:128], 128, cast=True)
                transpose_into(xb[0:32, :], xc1o[nb][:, 128:160], 32, cast=True)
                nc.vector.memset(xb[32:33, :], 1.0)
                xc1T_a[nb] = xa; xc1T_b[nb] = xb
            if sim:
                nc.sync.dma_start(out=xc1_tab[0:S, :], in_=dram_xc1[:, :])
            else:
                nc.gpsimd.collective_compute(
                    "AllGather", A.bypass, replica_groups=[list(range(NC))],
                    ins=[dram_xc1[:, :]], outs=[xc1_tab[:, :]])

            xc2o = bigconv(xc1_tab, w_s2, rs2a, rs2b, rts2a, rts2b, xc1T_a, xc1T_b, "s2")
            for nb in range(3):
                nc.sync.dma_start(out=xc2_out[nb * P:(nb + 1) * P, :], in_=xc2o[nb][:])

    nc.compile()
    return nc


def _prep(x, edge_index, edge_attr, batch, params):
    import ml_dtypes
    p = params
    src, dst = np.asarray(edge_index[0]), np.asarray(edge_index[1])
    x = np.asarray(x, np.float32)
    ea_np = np.asarray(edge_attr, np.float32)
    x2o = _host_cfc(x, p)

    order = np.argsort(dst, kind='stable')
    ssrc, sdst, sea = src[order], dst[order], ea_np[order]

    shards = []
    maxe = 0
    for s in range(NC):
        lo, hi = s * S, (s + 1) * S
        m = (sdst >= lo) & (sdst < hi)
        shards.append((ssrc[m], (sdst[m] - lo).astype(np.int64), sea[m]))
        maxe = max(maxe, int(m.sum()))
    EP = ((maxe + P - 1) // P) * P

    key = ("nc", EP)
    if key not in _cache:
        _cache[key] = _build(EP)
    nc = _cache[key]

    def pad_rows(a, n, fill=0.0):
        out = np.full((n,) + a.shape[1:], fill, np.float32)
        out[:a.shape[0]] = a
        return out

    xf = pad_rows(x, NP)
    x2of = pad_rows(np.asarray(x2o, np.float32), NP)
    x1f = np.ascontiguousarray(xf[:, :41])

    def aug(w, b):
        return np.concatenate([np.asarray(w, np.float32),
                               np.asarray(b, np.float32)[None, :]], 0)

    def perm_small(pp, cin):
        ew2 = np.asarray(pp['ew2'], np.float32)
        eb2 = np.asarray(pp['eb2'], np.float32)
        W = np.empty((33, 32 * cin), np.float32)
        w3 = ew2.reshape(32, cin, 32)
        W[0:32] = np.transpose(w3, (0, 2, 1)).reshape(32, 32 * cin)
        W[32] = eb2.reshape(cin, 32).T.reshape(-1)
        return W

    def perm_big(pp):
        ew2 = np.asarray(pp['ew2'], np.float32).reshape(32, HID, HID)
        eb2 = np.asarray(pp['eb2'], np.float32).reshape(HID, HID)
        R = np.empty((HID, 33 * HID), np.float32)
        for k in range(32):
            R[:, k * HID:(k + 1) * HID] = ew2[k]
        R[:, 32 * HID:] = eb2
        return R

    fca_in = {}
    for nm in ['inter', 'agg', 'aga', 'aae']:
        w = p[nm]
        fca_in[f"fca_{nm}_wq"] = aug(w['wq'], w['bq'])
        fca_in[f"fca_{nm}_wk"] = aug(w['wk'], w['bk'])
        fca_in[f"fca_{nm}_wv"] = aug(w['wv'], w['bv'])
        fca_in[f"fca_{nm}_wo"] = aug(w['wo'], w['bo'])

    rb_s1 = perm_big(p['sub1']); rb_s2 = perm_big(p['sub2'])
    bf = ml_dtypes.bfloat16

    in_maps = []
    for s in range(NC):
        es, ed, ea = shards[s]
        ne = len(es)
        srcg_f = np.zeros(EP, np.int32); srcg_f[:ne] = es
        srcg = np.ascontiguousarray(srcg_f.reshape(EP // P, P).T)
        dstl_f = np.full(EP, -1000.0, np.float32); dstl_f[:ne] = ed
        dstl = np.ascontiguousarray(dstl_f.reshape(EP // P, P).T)
        eaT_in = np.ones((11, EP), np.float32)
        eaT_in[0:10, :ne] = ea.T
        eaT_in[0:10, ne:] = 0.0
        deg = np.bincount(ed, minlength=S)[:S].astype(np.float32)
        rdeg_in = (1.0 / np.maximum(deg, 1.0)).reshape(3, P).T.copy()
        lo = s * S
        xo = xf[lo:lo + S]
        ones_row = np.ones((1, S), np.float32)
        m = dict(
            x2o_tab=x2of, x1_tab=x1f, eaT=eaT_in, srcg=srcg, dstl=dstl,
            iota=np.broadcast_to(np.arange(S, dtype=np.float32)[None, :], (P, S)).copy(),
            rdeg=rdeg_in,
            gT=np.concatenate([xo[:, 67:86].T, ones_row], 0),
            x3T=np.concatenate([xo[:, 48:67].T, ones_row], 0),
            x1T=np.concatenate([xo[:, 0:41].T, ones_row], 0),
            x2oT=np.concatenate([x2of[lo:lo + S].T, ones_row], 0),
            ew1b_a21=aug(p['a21']['ew1'], p['a21']['eb1']),
            ew1b_a11=aug(p['a11']['ew1'], p['a11']['eb1']),
            W2p_a21=perm_small(p['a21'], 32), W2p_a11=perm_small(p['a11'], 41),
            rootb_a21=aug(p['a21']['root'], p['a21']['bias']),
            rootb_a11=aug(p['a11']['root'], p['a11']['bias']),
            transw=aug(p['trans_w'], p['trans_b']),
            xmw_a=aug(p['xm_w'], p['xm_b'])[0:128].astype(bf),
            xmw_b=aug(p['xm_w'], p['xm_b'])[128:161].astype(bf),
            ew1b_s1=aug(p['sub1']['ew1'], p['sub1']['eb1']),
            ew1b_s2=aug(p['sub2']['ew1'], p['sub2']['eb1']),
            rhsb_s1a=rb_s1[0:128].astype(bf), rhsb_s1b=rb_s1[128:160].astype(bf),
            rhsb_s2a=rb_s2[0:128].astype(bf), rhsb_s2b=rb_s2[128:160].astype(bf),
            rootb_s1a=aug(p['sub1']['root'], p['sub1']['bias'])[0:128].astype(bf),
            rootb_s1b=aug(p['sub1']['root'], p['sub1']['bias'])[128:161].astype(bf),
            rootb_s2a=aug(p['sub2']['root'], p['sub2']['bias'])[0:128].astype(bf),
            rootb_s2b=aug(p['sub2']['root'], p['sub2']['bias'])[128:161].astype(bf),
            **{k: v.copy() for k, v in fca_in.items()},
        )
        in_maps.append(m)
    return nc, in_maps, ea_np


def kernel(x, edge_index, edge_attr, batch, params):
    import time
    from concourse.bass_utils import run_bass_kernel_spmd
    nc, in_maps, ea_np = _prep(x, edge_index, edge_attr, batch, params)
    t0 = time.time()
    res = run_bass_kernel_spmd(nc, in_maps, core_ids=list(range(NC)))
    dt_ns = int((time.time() - t0) * 1e9)
    _cache['exec_ns'] = min(_cache.get('exec_ns', 1 << 62), dt_ns)
    xc2 = np.concatenate([res.results[s]["xc2"] for s in range(NC)], 0)[:N]
    intf = np.concatenate([res.results[s]["interf"] for s in range(NC)], 0)[:N]
    src, dst = np.asarray(edge_index[0]), np.asarray(edge_index[1])
    y = _afp_tail(xc2, intf, src, dst, np.asarray(batch), ea_np, params)
    return y.astype(np.float32)


# revision 19
# speedup vs baseline: 1.0673x; 1.0252x over previous
"""MesoNet Trainium2 kernel: 8-core SPMD.

Device computes the dominant stages (NNConv a21/a11, 4x FCA+LN, trans/xm,
sub1/sub2 big edge-conditioned convs) with nodes + their incoming edges
sharded per core and AllGathers between conv layers.  The small graph-level
AttentiveFP tail (~2% of FLOPs) runs on host.
"""
import numpy as np

N, E, G = 3000, 6000, 150
NP = 3072
S = 384
NC = 8
HID = 160
P = 128

_cache = {}
ACT_PRED = lambda k: k % 3 != 0 and k < 31
SB_BUFS = 3
FCA_GPS = False
AP_GPS = False
CP_GPS = False


# ---------------- host math ----------------
def _sig(v):
    return 1.0 / (1.0 + np.exp(-v))


def _lrelu(v, a=0.01):
    return np.where(v >= 0, v, a * v)


def _elu(v):
    return np.where(v >= 0, v, np.expm1(v))


def _relu(v):
    return np.maximum(v, 0.0)


def _seg_sum(v, seg, n):
    out = np.zeros((n,) + v.shape[1:], np.float32)
    np.add.at(out, seg, v)
    return out


def _seg_softmax(a, seg, n):
    e = np.exp(a)
    s = _seg_sum(e, seg, n)
    return e / np.maximum(s[seg], 1e-16)


def _gru(xv, h, q):
    gi = xv @ q['wih'] + q['bih']
    gh = h @ q['whh'] + q['bhh']
    ir, iz, inn = np.split(gi, 3, -1)
    hr, hz, hn = np.split(gh, 3, -1)
    r = _sig(ir + hr)
    zt = _sig(iz + hz)
    nn_ = np.tanh(inn + r * hn)
    return (1 - zt) * nn_ + zt * h


def _gat(x_src, x_dst, src, dst, q, n_dst):
    hs = x_src @ q['w']
    hd = x_dst @ q['w']
    a = _lrelu((hs @ q['att_src'])[src] + (hd @ q['att_dst'])[dst])
    alpha = _seg_softmax(a, dst, n_dst)
    return _seg_sum(alpha[:, None] * hs[src], dst, n_dst) + q['bias']


def _afp_tail(xc, inter_f, src, dst, batch, edge_attr, p):
    q = p['afp']
    xv = _lrelu(xc @ q['lin1_w'] + q['lin1_b'])
    xj = _lrelu(np.concatenate([xv[src], edge_attr], -1) @ q['gate_lin1'])
    a = _lrelu(xj @ q['gate_att_l'] + (xv @ q['gate_att_r'])[dst])
    alpha = _seg_softmax(a, dst, N)
    hh = _seg_sum((xv[src] @ q['gate_lin2']) * alpha[:, None], dst, N) + q['gate_bias']
    xv = _relu(_gru(_elu(hh), xv, q['gru1']))
    hh = _elu(_gat(xv, xv, src, dst, q['conv1'], N))
    xv = _relu(_gru(hh, xv, q['gru2']))
    out = _relu(_seg_sum(xv, batch, G))
    row = np.arange(N)
    for _ in range(2):
        hh = _elu(_gat(xv, out, row, batch, q['mol_conv'], G))
        out = _relu(_gru(hh, out, q['mol_gru']))
    xg = out @ q['lin2_w'] + q['lin2_b']
    cnt = _seg_sum(np.ones(N, np.float32), batch, G)
    grp_pool = _relu((_seg_sum(inter_f, batch, G) / np.maximum(cnt, 1.0)[:, None])
                     @ p['group_w'] + p['group_b'])
    y = np.concatenate([xg, grp_pool], -1)
    y = _relu(y @ p['fc1_w'] + p['fc1_b'])
    y = _relu(y @ p['fc2_w'] + p['fc2_b'])
    y = _relu(y @ p['fc3_w'] + p['fc3_b'])
    return y @ p['fc4_w'] + p['fc4_b']


def _host_cfc(x, p):
    x2 = x[:, 42:48]
    lt = lambda v: 1.7159 * np.tanh(0.666 * v)
    c = p['cfc']
    h = np.concatenate([x2, x2], -1)
    outs = []
    for _ in range(5):
        z = np.concatenate([x2, h], -1)
        ti = _sig(z @ c['wta'] + c['bta'] + z @ c['wtb'] + c['btb'])
        h = lt(z @ c['wf1'] + c['bf1']) * (1.0 - ti) + ti * lt(z @ c['wf2'] + c['bf2'])
        outs.append(h[:, :6])
    return _relu(np.concatenate(outs, -1) @ p['x22_w'] + p['x22_b'])


# ---------------- device kernel ----------------
def _build(EP, sim=False):
    import concourse.bass as bass
    import concourse.mybir as mybir
    import concourse.tile as tile
    from concourse import bacc
    from concourse.masks import make_identity

    f32 = mybir.dt.float32
    bf16 = mybir.dt.bfloat16
    i32 = mybir.dt.int32
    A = mybir.AluOpType
    AF = mybir.ActivationFunctionType
    X = mybir.AxisListType.X
    NT = EP // P

    def bcast(ap, rep, axis_len):
        """[P, axis_len] -> [P, rep, axis_len] with stride-0 middle dim."""
        return bass.AP(ap.tensor, ap.offset, [ap.ap[0], [0, rep], ap.ap[1]])

    nc = bacc.Bacc("TRN2", target_bir_lowering=False, debug=False, num_devices=NC)

    def din(name, shape, dt=f32):
        return nc.dram_tensor(name, shape, dt, kind="ExternalInput")

    x2o_tab = din("x2o_tab", [NP, 32])
    x1_tab = din("x1_tab", [NP, 41])
    eaT = din("eaT", [11, EP])
    srcg = din("srcg", [P, EP // P], i32)
    dstl = din("dstl", [P, EP // P])
    iota = din("iota", [P, S])
    rdeg = din("rdeg", [P, 3])
    gT = din("gT", [20, S]); x3T = din("x3T", [20, S])
    x1T = din("x1T", [42, S]); x2oT = din("x2oT", [33, S])
    ew1b_a21 = din("ew1b_a21", [11, 32]); ew1b_a11 = din("ew1b_a11", [11, 32])
    W2p_a21 = din("W2p_a21", [33, 1024]); W2p_a11 = din("W2p_a11", [33, 1312])
    rootb_a21 = din("rootb_a21", [33, 32]); rootb_a11 = din("rootb_a11", [42, 32])
    fcaw = {}
    for nm, dq, dkv in [("inter", 19, 19), ("agg", 19, 32), ("aga", 32, 32), ("aae", 32, 32)]:
        fcaw[nm] = dict(
            wq=din(f"fca_{nm}_wq", [dq + 1, 32]), wk=din(f"fca_{nm}_wk", [dkv + 1, 32]),
            wv=din(f"fca_{nm}_wv", [dkv + 1, 32]), wo=din(f"fca_{nm}_wo", [33, 32]))
    transw = din("transw", [97, 96])
    xmw_a = din("xmw_a", [128, 160], bf16)
    xmw_b = din("xmw_b", [33, 160], bf16)
    ew1b_s1 = din("ew1b_s1", [11, 32]); ew1b_s2 = din("ew1b_s2", [11, 32])
    rhsb_s1a = din("rhsb_s1a", [128, 5280], bf16); rhsb_s1b = din("rhsb_s1b", [32, 5280], bf16)
    rhsb_s2a = din("rhsb_s2a", [128, 5280], bf16); rhsb_s2b = din("rhsb_s2b", [32, 5280], bf16)
    rootb_s1a = din("rootb_s1a", [128, 160], bf16); rootb_s1b = din("rootb_s1b", [33, 160], bf16)
    rootb_s2a = din("rootb_s2a", [128, 160], bf16); rootb_s2b = din("rootb_s2b", [33, 160], bf16)

    xc2_out = nc.dram_tensor("xc2", [S, HID], f32, kind="ExternalOutput")
    intf_out = nc.dram_tensor("interf", [S, 32], f32, kind="ExternalOutput")

    with tile.TileContext(nc) as tc:
        with (
            tc.tile_pool(name="sb", bufs=SB_BUFS) as sb,
            tc.tile_pool(name="keep", bufs=1) as kp,
            tc.tile_pool(name="ps", bufs=PS_BUFS, space="PSUM") as ps,
            tc.tile_pool(name="pw", bufs=2, space="PSUM") as pw,
            tc.tile_pool(name="psagg", bufs=1, space="PSUM") as psa,
            tc.tile_pool(name="dram", bufs=1, space="DRAM") as dram,
        ):
            dram_xc0 = dram.tile([S, HID], f32, name="dram_xc0")
            xc_tab = dram.tile([NP, HID], f32, name="xc_tab", addr_space="Shared")
            dram_xc1 = dram.tile([S, HID], f32, name="dram_xc1")
            xc1_tab = dram.tile([NP, HID], f32, name="xc1_tab", addr_space="Shared")

            ident = kp.tile([P, P], f32, name="ident")
            make_identity(nc, ident[:])
            eps_t = kp.tile([P, 1], f32, name="eps_t")
            nc.vector.memset(eps_t[:], 1e-5)
            identb = kp.tile([P, P], bf16, name="identb")
            nc.vector.tensor_copy(identb[:], ident[:])

            def load(t, dt=f32):
                tl = kp.tile(list(t.shape), dt, name="ld_" + t.name)
                nc.sync.dma_start(out=tl[:], in_=t[:, :])
                return tl

            iota_t = load(iota); dstl_t = load(dstl); rdeg_t = load(rdeg)
            eaT_t = load(eaT)
            gT_t = load(gT); x3T_t = load(x3T); x1T_t = load(x1T); x2oT_t = load(x2oT)
            w_a21 = load(ew1b_a21); w_a11 = load(ew1b_a11)
            W2a21 = load(W2p_a21); W2a11 = load(W2p_a11)
            rb21 = load(rootb_a21); rb11 = load(rootb_a11)
            transw_t = load(transw)
            xmw_ta = load(xmw_a, bf16); xmw_tb = load(xmw_b, bf16)
            w_s1 = load(ew1b_s1); w_s2 = load(ew1b_s2)
            rs1a = load(rhsb_s1a, bf16); rs1b = load(rhsb_s1b, bf16)
            rs2a = load(rhsb_s2a, bf16); rs2b = load(rhsb_s2b, bf16)
            rts1a = load(rootb_s1a, bf16); rts1b = load(rootb_s1b, bf16)
            rts2a = load(rootb_s2a, bf16); rts2b = load(rootb_s2b, bf16)
            fcawt = {k: {kk: load(vv) for kk, vv in v.items()} for k, v in fcaw.items()}
            src_t = load(srcg, i32)

            Aps = {}
            for et in range(NT):
                for nb in range(3):
                    ap = kp.tile([P, P], f32, name=f"Ap_{et}_{nb}")
                    (nc.gpsimd if AP_GPS else nc.vector).tensor_scalar(
                        out=ap[:], in0=iota_t[:, nb * P:(nb + 1) * P],
                        scalar1=dstl_t[:, et:et + 1], scalar2=None,
                        op0=A.is_equal)
                    Aps[(et, nb)] = ap

            def transpose_into(dst_ap, src_ap, cols, cast=False):
                pt = ps.tile([P, P], f32, name="tp", tag="tp")
                nc.tensor.transpose(out=pt[0:cols, :], in_=src_ap, identity=ident[:])
                if cast:
                    nc.vector.tensor_copy(dst_ap, pt[0:cols, :])
                else:
                    nc.scalar.copy(dst_ap, pt[0:cols, :])

            # ---------- phase 1: a21 + a11 ----------
            agg_s = [psa.tile([P, 512], f32, name=f"agg{nb}", tag=f"agg{nb}")
                     for nb in range(3)]
            for et in range(NT):
                esl = slice(et * P, (et + 1) * P)
                msgs = sb.tile([P, 64], f32, name="msgs")
                for (wname, W2, cin, co, tab) in [
                        ("a21", W2a21, 32, 0, x2o_tab), ("a11", W2a11, 41, 32, x1_tab)]:
                    wt = w_a21 if wname == "a21" else w_a11
                    h1p = ps.tile([32, P], f32, name="h1p", tag="tp")
                    nc.tensor.matmul(h1p[0:32, :], lhsT=wt[:], rhs=eaT_t[:, esl],
                                     start=True, stop=True)
                    h1T = sb.tile([33, P], f32, name="h1T_" + wname)
                    nc.scalar.activation(h1T[0:32, :], h1p[0:32, :], AF.Relu)
                    nc.vector.memset(h1T[32:33, :], 1.0)
                    xs = sb.tile([P, 48], f32, name="xs_s")
                    nc.gpsimd.indirect_dma_start(
                        out=xs[:, 0:cin], out_offset=None, in_=tab[:, :],
                        in_offset=bass.IndirectOffsetOnAxis(ap=src_t[:, et:et + 1], axis=0))
                    oc = 512 // cin
                    for o0 in range(0, 32, oc):
                        no = min(oc, 32 - o0)
                        wid = no * cin
                        wps = pw.tile([P, 512], f32, name="wps", tag="wps")
                        nc.tensor.matmul(wps[:, 0:wid], lhsT=h1T[:],
                                         rhs=W2[:, o0 * cin:(o0 + no) * cin],
                                         start=True, stop=True)
                        tmp = sb.tile([P, 512], bf16, name="tmp_e")
                        nc.vector.tensor_tensor(
                            out=tmp[:, 0:wid].rearrange("p (o i) -> p o i", i=cin),
                            in0=wps[:, 0:wid].rearrange("p (o i) -> p o i", i=cin),
                            in1=bcast(xs[:, 0:cin], no, cin), op=A.mult)
                        nc.vector.tensor_reduce(
                            out=msgs[:, co + o0:co + o0 + no],
                            in_=tmp[:, 0:wid].rearrange("p (o i) -> p o i", i=cin),
                            axis=X, op=A.add)
                for nb in range(3):
                    nc.tensor.matmul(agg_s[nb][:, 0:64], lhsT=Aps[(et, nb)][:],
                                     rhs=msgs[:], start=(et == 0), stop=(et == NT - 1))

            x1v = {}; x2v = {}; x1vT = {}; x2vT = {}
            for nb in range(3):
                nsl = slice(nb * P, (nb + 1) * P)
                nc.tensor.matmul(agg_s[nb][:, 64:96], lhsT=x2oT_t[:, nsl], rhs=rb21[:],
                                 start=True, stop=True)
                nc.tensor.matmul(agg_s[nb][:, 96:128], lhsT=x1T_t[:, nsl], rhs=rb11[:],
                                 start=True, stop=True)
                for (c0, r0, dd) in [(0, 64, x2v), (32, 96, x1v)]:
                    sA = sb.tile([P, 32], f32, name="sA")
                    nc.vector.tensor_scalar(out=sA[:], in0=agg_s[nb][:, c0:c0 + 32],
                                            scalar1=rdeg_t[:, nb:nb + 1], scalar2=None,
                                            op0=A.mult)
                    vv = kp.tile([P, 32], f32, name=f"v{r0}_{nb}")
                    nc.vector.tensor_tensor(out=vv[:], in0=sA[:],
                                            in1=agg_s[nb][:, r0:r0 + 32], op=A.add)
                    nc.vector.tensor_scalar(out=vv[:], in0=vv[:], scalar1=0.0,
                                            scalar2=None, op0=A.max)
                    dd[nb] = vv
                for (dd, dt_, pref) in [(x2v, x2vT, "t2v"), (x1v, x1vT, "t1v")]:
                    tt = kp.tile([33, P], f32, name=f"{pref}_{nb}")
                    transpose_into(tt[0:32, :], dd[nb][:], 32)
                    nc.vector.memset(tt[32:33, :], 1.0)
                    dt_[nb] = tt

            # ---------- phase 2: FCA chain ----------
            def fca_tile(qT_ap, kvT_ap, w, nb, name):
                qkv = psa.tile([P, 96], f32, name=f"qkv_{name}_{nb}", tag=f"agg{nb}")
                nc.tensor.matmul(qkv[:, 0:32], lhsT=qT_ap, rhs=w['wq'][:], start=True, stop=True)
                nc.tensor.matmul(qkv[:, 32:64], lhsT=kvT_ap, rhs=w['wk'][:], start=True, stop=True)
                nc.tensor.matmul(qkv[:, 64:96], lhsT=kvT_ap, rhs=w['wv'][:], start=True, stop=True)
                q_s = sb.tile([P, 32], f32, name="q_s")
                k_s = sb.tile([P, 32], f32, name="k_s")
                v_s = sb.tile([P, 32], bf16, name="v_s")
                nc.scalar.copy(q_s[:], qkv[:, 0:32])
                nc.scalar.copy(k_s[:], qkv[:, 32:64])
                nc.vector.tensor_copy(v_s[:], qkv[:, 64:96])
                lg = sb.tile([P, 1024], f32, name="lg")
                qb = bass.AP(q_s[:].tensor, q_s[:].offset,
                             [q_s[:].ap[0], [1, 32], [0, 32]])
                (nc.gpsimd if FCA_GPS else nc.vector).tensor_tensor(
                    out=lg[:].rearrange("p (i j) -> p i j", j=32),
                    in0=qb, in1=bcast(k_s[:, 0:32], 32, 32), op=A.mult)
                ee = sb.tile([P, 1024], bf16, name="ee")
                nc.scalar.activation(ee[:], lg[:], AF.Exp)
                ev = sb.tile([P, 1024], bf16, name="ev")
                (nc.gpsimd if FCA_GPS else nc.vector).tensor_tensor(
                    out=ev[:].rearrange("p (i j) -> p i j", j=32),
                    in0=ee[:].rearrange("p (i j) -> p i j", j=32),
                    in1=bcast(v_s[:, 0:32], 32, 32), op=A.mult)
                o_u = sb.tile([P, 32], f32, name="o_u")
                s_u = sb.tile([P, 32], f32, name="s_u")
                nc.vector.tensor_reduce(out=o_u[:], in_=ev[:].rearrange("p (i j) -> p i j", j=32),
                                        axis=X, op=A.add)
                nc.vector.tensor_reduce(out=s_u[:], in_=ee[:].rearrange("p (i j) -> p i j", j=32),
                                        axis=X, op=A.add)
                rs = sb.tile([P, 32], f32, name="rs")
                nc.vector.reciprocal(rs[:], s_u[:])
                oo = sb.tile([P, 32], f32, name="oo")
                nc.vector.tensor_tensor(out=oo[:], in0=o_u[:], in1=rs[:], op=A.mult)
                ooT = sb.tile([33, P], f32, name="ooT")
                transpose_into(ooT[0:32, :], oo[:], 32)
                nc.vector.memset(ooT[32:33, :], 1.0)
                op_ = pw.tile([P, 512], f32, name="wps", tag="wps")
                nc.tensor.matmul(op_[:, 0:32], lhsT=ooT[:], rhs=w['wo'][:], start=True, stop=True)
                z = sb.tile([P, 32], f32, name="z_ln", tag=f"z_ln{nb}")
                nc.vector.tensor_tensor(out=z[:], in0=q_s[:], in1=op_[:, 0:32], op=A.add)
                return z

            def ln_finish(zs, name):
                zcs = {}; vss = {}; rsqs = {}
                for nb, z in zs.items():
                    mu = sb.tile([P, 1], f32, name="mu")
                    nc.vector.tensor_reduce(out=mu[:], in_=z[:], axis=X, op=A.add)
                    nc.vector.tensor_scalar(out=mu[:], in0=mu[:], scalar1=1.0 / 32,
                                            scalar2=None, op0=A.mult)
                    zc = sb.tile([P, 32], f32, name="zc", tag=f"zc{nb}")
                    nc.vector.tensor_scalar(out=zc[:], in0=z[:], scalar1=mu[:, 0:1],
                                            scalar2=None, op0=A.subtract)
                    junk = sb.tile([P, 32], bf16, name="junk")
                    vs = sb.tile([P, 1], f32, name="vs", tag=f"vs{nb}")
                    nc.scalar.activation(junk[:], zc[:], AF.Square, accum_out=vs[:])
                    zcs[nb] = zc; vss[nb] = vs
                for nb in zs:
                    lnv = sb.tile([P, 1], f32, name="lnv", tag=f"lnv{nb}")
                    nc.scalar.activation(lnv[:], vss[nb][:], AF.Ln, scale=1.0 / 32,
                                         bias=eps_t[:, 0:1])
                    rsqs[nb] = lnv
                for nb in zs:
                    rsq = sb.tile([P, 1], f32, name="rsq", tag=f"rsq{nb}")
                    nc.scalar.activation(rsq[:], rsqs[nb][:], AF.Exp, scale=-0.5)
                    rsqs[nb] = rsq
                outs = {}
                for nb in zs:
                    out = kp.tile([P, 32], f32, name=f"fca_{name}_{nb}")
                    nc.vector.tensor_scalar(out=out[:], in0=zcs[nb][:],
                                            scalar1=rsqs[nb][:, 0:1],
                                            scalar2=None, op0=A.mult)
                    outT = kp.tile([33, P], f32, name=f"fcaT_{name}_{nb}")
                    transpose_into(outT[0:32, :], out[:], 32)
                    nc.vector.memset(outT[32:33, :], 1.0)
                    outs[nb] = (out, outT)
                return outs

            xcT_a = {}; xcT_b = {}
            zs = {nb: fca_tile(gT_t[:, nb * P:(nb + 1) * P],
                               gT_t[:, nb * P:(nb + 1) * P], fcawt['inter'], nb, "in")
                  for nb in range(3)}
            interR = ln_finish(zs, "in")
            zs = {nb: fca_tile(x3T_t[:, nb * P:(nb + 1) * P], interR[nb][1][:],
                               fcawt['agg'], nb, "gu") for nb in range(3)}
            guR = ln_finish(zs, "gu")
            zs = {nb: fca_tile(x1vT[nb][:], guR[nb][1][:], fcawt['aga'], nb, "gr")
                  for nb in range(3)}
            grpR = ln_finish(zs, "gr")
            zs = {nb: fca_tile(x2vT[nb][:], grpR[nb][1][:], fcawt['aae'], nb, "au")
                  for nb in range(3)}
            auR = ln_finish(zs, "au")
            for nb in range(3):
                nsl = slice(nb * P, (nb + 1) * P)
                inter_o, interT = interR[nb]
                gu_o, guT = guR[nb]
                grp_o, grpT = grpR[nb]
                au_o, auT = auR[nb]
                nc.sync.dma_start(out=intf_out[nsl, :], in_=inter_o[:])
                catT = sb.tile([97, P], f32, name="catT")
                (nc.gpsimd if CP_GPS else nc.vector).tensor_copy(catT[0:32, :], guT[0:32, :])
                (nc.gpsimd if CP_GPS else nc.vector).tensor_copy(catT[32:64, :], grpT[0:32, :])
                (nc.gpsimd if CP_GPS else nc.vector).tensor_copy(catT[64:96, :], auT[0:32, :])
                nc.vector.memset(catT[96:97, :], 1.0)
                xxp = pw.tile([P, 512], f32, name="wps", tag="wps")
                nc.tensor.matmul(xxp[:, 0:96], lhsT=catT[:], rhs=transw_t[:],
                                 start=True, stop=True)
                xx = sb.tile([P, 96], f32, name="xx")
                nc.scalar.activation(xx[:], xxp[:, 0:96], AF.Relu)
                xcat_a = sb.tile([P, P], bf16, name="xcat_a")
                xcat_b = sb.tile([33, P], bf16, name="xcat_b")
                (nc.gpsimd if CP_GPS else nc.vector).tensor_copy(xcat_a[0:32, :], x1vT[nb][0:32, :])
                (nc.gpsimd if CP_GPS else nc.vector).tensor_copy(xcat_a[32:64, :], x2vT[nb][0:32, :])
                ptx = ps.tile([P, P], f32, name="tp", tag="tp")
                nc.tensor.transpose(out=ptx[0:96, :], in_=xx[:], identity=ident[:])
                nc.vector.tensor_copy(xcat_a[64:128, :], ptx[0:64, :])
                nc.vector.tensor_copy(xcat_b[0:32, :], ptx[64:96, :])
                nc.vector.memset(xcat_b[32:33, :], 1.0)
                xcp = pw.tile([P, 512], f32, name="wps", tag="wps")
                nc.tensor.matmul(xcp[:, 0:160], lhsT=xcat_a[:], rhs=xmw_ta[:],
                                 start=True, stop=False)
                nc.tensor.matmul(xcp[:, 0:160], lhsT=xcat_b[:], rhs=xmw_tb[:],
                                 start=False, stop=True)
                xc = sb.tile([P, HID], f32, name="xc")
                nc.scalar.activation(xc[:], xcp[:, 0:160], AF.Relu)
                xa = kp.tile([P, P], bf16, name=f"xcTa{nb}")
                xb = kp.tile([33, P], bf16, name=f"xcTb{nb}")
                transpose_into(xa[:], xc[:, 0:128], 128, cast=True)
                transpose_into(xb[0:32, :], xc[:, 128:160], 32, cast=True)
                nc.vector.memset(xb[32:33, :], 1.0)
                xcT_a[nb] = xa; xcT_b[nb] = xb
                nc.sync.dma_start(out=dram_xc0[nsl, :], in_=xc[:])

            if sim:
                nc.sync.dma_start(out=xc_tab[0:S, :], in_=dram_xc0[:, :])
            else:
                nc.gpsimd.collective_compute(
                    "AllGather", A.bypass, replica_groups=[list(range(NC))],
                    ins=[dram_xc0[:, :]], outs=[xc_tab[:, :]])

            # ---------- big convs ----------
            def bigconv(tab, w_e, rs_a, rs_b, rtwa, rtwb, xTa, xTb, stage):
                agg = [psa.tile([P, 512], f32, name=f"agg{nb}_{stage}", tag=f"agg{nb}")
                       for nb in range(3)]
                msg2s = []
                for et in range(NT):
                    esl = slice(et * P, (et + 1) * P)
                    h1p = ps.tile([P, 96], f32, name="h1pe", tag="tp")
                    nc.tensor.matmul(h1p[:, 0:32], lhsT=eaT_t[:, esl], rhs=w_e[:],
                                     start=True, stop=True)
                    h1 = sb.tile([P, 33], f32, name="h1e")
                    nc.scalar.activation(h1[:, 0:32], h1p[:, 0:32], AF.Relu)
                    nc.vector.memset(h1[:, 32:33], 1.0)
                    xs = sb.tile([P, HID], f32, name="xs_b")
                    nc.gpsimd.indirect_dma_start(
                        out=xs[:], out_offset=None, in_=tab[:, :],
                        in_offset=bass.IndirectOffsetOnAxis(ap=src_t[:, et:et + 1], axis=0))
                    xsTa = sb.tile([P, P], bf16, name="xsTa")
                    xsTb = sb.tile([32, P], bf16, name="xsTb")
                    transpose_into(xsTa[:], xs[:, 0:128], 128, cast=True)
                    transpose_into(xsTb[0:32, :], xs[:, 128:160], 32, cast=True)
                    msg = sb.tile([P, HID], f32, name="msg_b")
                    msgp = ps.tile([P, HID], f32, name="msgp", tag="tp")
                    nacts = 0
                    for kc in range(11):
                        c0 = kc * 480
                        tps = pw.tile([P, 512], f32, name="wps", tag="wps")
                        nc.tensor.matmul(tps[:, 0:480], lhsT=xsTa[:],
                                         rhs=rs_a[:, c0:c0 + 480], start=True, stop=False)
                        nc.tensor.matmul(tps[:, 0:480], lhsT=xsTb[:],
                                         rhs=rs_b[:, c0:c0 + 480], start=False, stop=True)
                        for j in range(3):
                            k = kc * 3 + j
                            if k == 0:
                                nc.vector.tensor_scalar(
                                    out=msg[:], in0=tps[:, 0:160],
                                    scalar1=h1[:, 0:1], scalar2=None, op0=A.mult)
                            elif ACT_PRED(k):
                                tmpk = sb.tile([P, HID], bf16, name="tmpk")
                                nc.scalar.activation(tmpk[:], tps[:, j * 160:(j + 1) * 160],
                                                     AF.Copy, scale=h1[:, k:k + 1])
                                nc.tensor.matmul(msgp[:], lhsT=identb[:], rhs=tmpk[:],
                                                 start=(nacts == 0), stop=False)
                                nacts += 1
                            else:
                                nc.vector.scalar_tensor_tensor(
                                    out=msg[:], in0=tps[:, j * 160:(j + 1) * 160],
                                    scalar=h1[:, k:k + 1], in1=msg[:],
                                    op0=A.mult, op1=A.add)
                    msg2 = kp.tile([P, HID], f32, name=f"msg2_{stage}_{et}")
                    nc.vector.tensor_tensor(out=msg2[:], in0=msgp[:], in1=msg[:], op=A.add)
                    msg2s.append(msg2)
                for et in range(NT):
                    for nb in range(3):
                        nc.tensor.matmul(agg[nb][:, 0:160], lhsT=Aps[(et, nb)][:],
                                         rhs=msg2s[et][:], start=(et == 0), stop=(et == NT - 1))
                outs = []
                for nb in range(3):
                    nc.tensor.matmul(agg[nb][:, 160:320], lhsT=xTa[nb][:],
                                     rhs=rtwa[:], start=True, stop=False)
                    nc.tensor.matmul(agg[nb][:, 160:320], lhsT=xTb[nb][:],
                                     rhs=rtwb[:], start=False, stop=True)
                    sA = sb.tile([P, HID], f32, name="sAb")
                    nc.vector.tensor_scalar(out=sA[:], in0=agg[nb][:, 0:160],
                                            scalar1=rdeg_t[:, nb:nb + 1], scalar2=None,
                                            op0=A.mult)
                    oo = sb.tile([P, HID], f32, name="oo_b")
                    nc.vector.tensor_tensor(out=oo[:], in0=sA[:],
                                            in1=agg[nb][:, 160:320], op=A.add)
                    o = kp.tile([P, HID], f32, name=f"xcs_{stage}_{nb}")
                    nc.scalar.activation(o[:], oo[:], AF.Relu)
                    outs.append(o)
                return outs

            xc1o = bigconv(xc_tab, w_s1, rs1a, rs1b, rts1a, rts1b, xcT_a, xcT_b, "s1")
            xc1T_a = {}; xc1T_b = {}
            for nb in range(3):
                nc.sync.dma_start(out=dram_xc1[nb * P:(nb + 1) * P, :], in_=xc1o[nb][:])
                xa = kp.tile([P, P], bf16, name=f"x1Ta{nb}")
                xb = kp.tile([33, P], bf16, name=f"x1Tb{nb}")
                transpose_into(xa[:], xc1o[nb][:, 0:128], 128, cast=True)
                transpose_into(xb[0:32, :], xc1o[nb][:, 128:160], 32, cast=True)
                nc.vector.memset(xb[32:33, :], 1.0)
                xc1T_a[nb] = xa; xc1T_b[nb] = xb
            if sim:
                nc.sync.dma_start(out=xc1_tab[0:S, :], in_=dram_xc1[:, :])
            else:
                nc.gpsimd.collective_compute(
                    "AllGather", A.bypass, replica_groups=[list(range(NC))],
                    ins=[dram_xc1[:, :]], outs=[xc1_tab[:, :]])

            xc2o = bigconv(xc1_tab, w_s2, rs2a, rs2b, rts2a, rts2b, xc1T_a, xc1T_b, "s2")
            for nb in range(3):
                nc.sync.dma_start(out=xc2_out[nb * P:(nb + 1) * P, :], in_=xc2o[nb][:])

    nc.compile()
    return nc


def _prep(x, edge_index, edge_attr, batch, params):
    import ml_dtypes
    p = params
    src, dst = np.asarray(edge_index[0]), np.asarray(edge_index[1])
    x = np.asarray(x, np.float32)
    ea_np = np.asarray(edge_attr, np.float32)
    x2o = _host_cfc(x, p)

    order = np.argsort(dst, kind='stable')
    ssrc, sdst, sea = src[order], dst[order], ea_np[order]

    shards = []
    maxe = 0
    for s in range(NC):
        lo, hi = s * S, (s + 1) * S
        m = (sdst >= lo) & (sdst < hi)
        shards.append((ssrc[m], (sdst[m] - lo).astype(np.int64), sea[m]))
        maxe = max(maxe, int(m.sum()))
    EP = ((maxe + P - 1) // P) * P

    key = ("nc", EP)
    if key not in _cache:
        _cache[key] = _build(EP)
    nc = _cache[key]

    def pad_rows(a, n, fill=0.0):
        out = np.full((n,) + a.shape[1:], fill, np.float32)
        out[:a.shape[0]] = a
        return out

    xf = pad_rows(x, NP)
    x2of = pad_rows(np.asarray(x2o, np.float32), NP)
    x1f = np.ascontiguousarray(xf[:, :41])

    def aug(w, b):
        return np.concatenate([np.asarray(w, np.float32),
                               np.asarray(b, np.float32)[None, :]], 0)

    def perm_small(pp, cin):
        ew2 = np.asarray(pp['ew2'], np.float32)
        eb2 = np.asarray(pp['eb2'], np.float32)
        W = np.empty((33, 32 * cin), np.float32)
        w3 = ew2.reshape(32, cin, 32)
        W[0:32] = np.transpose(w3, (0, 2, 1)).reshape(32, 32 * cin)
        W[32] = eb2.reshape(cin, 32).T.reshape(-1)
        return W

    def perm_big(pp):
        ew2 = np.asarray(pp['ew2'], np.float32).reshape(32, HID, HID)
        eb2 = np.asarray(pp['eb2'], np.float32).reshape(HID, HID)
        R = np.empty((HID, 33 * HID), np.float32)
        for k in range(32):
            R[:, k * HID:(k + 1) * HID] = ew2[k]
        R[:, 32 * HID:] = eb2
        return R

    fca_in = {}
    for nm in ['inter', 'agg', 'aga', 'aae']:
        w = p[nm]
        fca_in[f"fca_{nm}_wq"] = aug(w['wq'], w['bq'])
        fca_in[f"fca_{nm}_wk"] = aug(w['wk'], w['bk'])
        fca_in[f"fca_{nm}_wv"] = aug(w['wv'], w['bv'])
        fca_in[f"fca_{nm}_wo"] = aug(w['wo'], w['bo'])

    rb_s1 = perm_big(p['sub1']); rb_s2 = perm_big(p['sub2'])
    bf = ml_dtypes.bfloat16

    in_maps = []
    for s in range(NC):
        es, ed, ea = shards[s]
        ne = len(es)
        srcg_f = np.zeros(EP, np.int32); srcg_f[:ne] = es
        srcg = np.ascontiguousarray(srcg_f.reshape(EP // P, P).T)
        dstl_f = np.full(EP, -1000.0, np.float32); dstl_f[:ne] = ed
        dstl = np.ascontiguousarray(dstl_f.reshape(EP // P, P).T)
        eaT_in = np.ones((11, EP), np.float32)
        eaT_in[0:10, :ne] = ea.T
        eaT_in[0:10, ne:] = 0.0
        deg = np.bincount(ed, minlength=S)[:S].astype(np.float32)
        rdeg_in = (1.0 / np.maximum(deg, 1.0)).reshape(3, P).T.copy()
        lo = s * S
        xo = xf[lo:lo + S]
        ones_row = np.ones((1, S), np.float32)
        m = dict(
            x2o_tab=x2of, x1_tab=x1f, eaT=eaT_in, srcg=srcg, dstl=dstl,
            iota=np.broadcast_to(np.arange(S, dtype=np.float32)[None, :], (P, S)).copy(),
            rdeg=rdeg_in,
            gT=np.concatenate([xo[:, 67:86].T, ones_row], 0),
            x3T=np.concatenate([xo[:, 48:67].T, ones_row], 0),
            x1T=np.concatenate([xo[:, 0:41].T, ones_row], 0),
            x2oT=np.concatenate([x2of[lo:lo + S].T, ones_row], 0),
            ew1b_a21=aug(p['a21']['ew1'], p['a21']['eb1']),
            ew1b_a11=aug(p['a11']['ew1'], p['a11']['eb1']),
            W2p_a21=perm_small(p['a21'], 32), W2p_a11=perm_small(p['a11'], 41),
            rootb_a21=aug(p['a21']['root'], p['a21']['bias']),
            rootb_a11=aug(p['a11']['root'], p['a11']['bias']),
            transw=aug(p['trans_w'], p['trans_b']),
            xmw_a=aug(p['xm_w'], p['xm_b'])[0:128].astype(bf),
            xmw_b=aug(p['xm_w'], p['xm_b'])[128:161].astype(bf),
            ew1b_s1=aug(p['sub1']['ew1'], p['sub1']['eb1']),
            ew1b_s2=aug(p['sub2']['ew1'], p['sub2']['eb1']),
            rhsb_s1a=rb_s1[0:128].astype(bf), rhsb_s1b=rb_s1[128:160].astype(bf),
            rhsb_s2a=rb_s2[0:128].astype(bf), rhsb_s2b=rb_s2[128:160].astype(bf),
            rootb_s1a=aug(p['sub1']['root'], p['sub1']['bias'])[0:128].astype(bf),
            rootb_s1b=aug(p['sub1']['root'], p['sub1']['bias'])[128:161].astype(bf),
            rootb_s2a=aug(p['sub2']['root'], p['sub2']['bias'])[0:128].astype(bf),
            rootb_s2b=aug(p['sub2']['root'], p['sub2']['bias'])[128:161].astype(bf),
            **{k: v.copy() for k, v in fca_in.items()},
        )
        in_maps.append(m)
    return nc, in_maps, ea_np


def kernel(x, edge_index, edge_attr, batch, params):
    import time
    from concourse.bass_utils import run_bass_kernel_spmd
    nc, in_maps, ea_np = _prep(x, edge_index, edge_attr, batch, params)
    t0 = time.time()
    res = run_bass_kernel_spmd(nc, in_maps, core_ids=list(range(NC)))
    dt_ns = int((time.time() - t0) * 1e9)
    _cache['exec_ns'] = min(_cache.get('exec_ns', 1 << 62), dt_ns)
    xc2 = np.concatenate([res.results[s]["xc2"] for s in range(NC)], 0)[:N]
    intf = np.concatenate([res.results[s]["interf"] for s in range(NC)], 0)[:N]
    src, dst = np.asarray(edge_index[0]), np.asarray(edge_index[1])
    y = _afp_tail(xc2, intf, src, dst, np.asarray(batch), ea_np, params)
    return y.astype(np.float32)
